# revision 1
# baseline (speedup 1.0000x reference)
"""Trainium2 Bass kernel for nn_EncoderLayer (B=4, S=1024, D=1024, H=16, FF=2048).

Sharding: 8 cores = 4 batches x 2 sequence-halves. Each core redundantly
computes K/V projections for its whole batch (no collectives) and runs the
full layer for its own 512 query rows. Odd cores receive the sequence
rotated by 512 so local queries are always columns 0:512 (softmax over keys
is permutation-invariant, so K/V order doesn't matter).

On-chip layout is feature-major (activations stored transposed, [feature,
token]), which makes every matmul in the layer transpose-free:
  - weights ([d_in, d_out] in DRAM) are directly the stationary lhsT operand
  - per-feature biases / layernorm gamma,beta are per-partition scalars
  - softmax normalization (Z) comes free from a ones-augmented V column
  - layernorm stats are column sums computed with ones-vector matmuls
Matmuls run in float32r (fp32 storage, TF32-like PE mode, 1 cycle/row).

Attention processes heads in pairs (even head in PE rows 0:63, odd head in
rows 64:127 -> concurrent matmuls via row tiling), with one [128,1024] exp
per (pair, key-tile) and softmax normalization deferred off the PE critical
path (fast approximate reciprocal + gpsimd partition broadcast).

The host transposes x per batch on the way in and the output back on the
way out (both cheap numpy ops), so the device never transposes anything.
"""

import sys
import types

import numpy as np


def _shim_axon_hooks():
    """bass_utils imports antenv.axon_hooks in its trace path; the module is
    absent from this image. Provide a no-op stand-in (only used when tracing)."""
    try:
        import antenv.axon_hooks  # noqa: F401
    except Exception:
        mod = types.ModuleType("antenv.axon_hooks")
        mod.get_axon_ntff_profile_hook = lambda: None
        mod.set_axon_ntff_profile_hook = lambda h: None
        sys.modules["antenv.axon_hooks"] = mod


_shim_axon_hooks()

from concourse import bacc, mybir, tile  # noqa: E402
from concourse import bass_utils  # noqa: E402

F32 = mybir.dt.float32
F32R = mybir.dt.float32r
BF16 = mybir.dt.bfloat16
AF = mybir.ActivationFunctionType

B, S, D, H, DH, FF = 4, 1024, 1024, 16, 64, 2048
SQ = 512          # query rows per core
P = 128
DT = D // P       # 8 d_model tiles
FT = FF // P      # 16 ffn tiles
ST = S // P       # 8 key tiles
NCORES = 8
EPS = 1e-6
SCALE = 1.0 / 32.0  # 1/sqrt(D_MODEL)

# consts layout (one [128, 64] f32 array, column ranges):
_C_BO = 0    # 8 cols: bo per d-tile
_C_B1 = 8    # 16 cols: b1 per f-tile
_C_B2 = 24   # 8 cols
_C_G1 = 32   # 8 cols
_C_BE1 = 40  # 8 cols
_C_G2 = 48   # 8 cols
_C_BE2 = 56  # 8 cols


def _emit(ctx, tc, aps):
    nc = tc.nc
    xT_ap, wq_ap, wk_ap, wv_ap, wo_ap, w1_ap, w2_ap, consts_ap, ones_ap, onesrow_ap, fold_ap, yT_ap = aps

    acts = ctx.enter_context(tc.tile_pool(name="acts", bufs=1))
    wf = ctx.enter_context(tc.tile_pool(name="wf", bufs=7))
    sc2 = ctx.enter_context(tc.tile_pool(name="sc2", bufs=2))
    sc1 = ctx.enter_context(tc.tile_pool(name="sc1", bufs=1))
    pp = ctx.enter_context(tc.tile_pool(name="pp", bufs=2, space="PSUM"))
    pvp = ctx.enter_context(tc.tile_pool(name="pvp", bufs=4, space="PSUM"))

    def wslice(pool, src_ap, nk_off, col_off):
        """Load a [P, 8, P] stationary-weight tile: 8 contraction k-tiles of
        one 128-wide output column block. Split into 4 dma_starts so the
        512KB transfer spreads over 4 DMA queues (a single queue is only
        ~41GB/s -> 12us; split -> ~3us)."""
        w = pool.tile([P, 8, P], F32R, tag="w", name="w")
        src = src_ap.rearrange("(k p) n -> p k n", p=P)
        for q in range(4):
            nc.sync.dma_start(
                w[:, 2 * q : 2 * q + 2, :],
                src[:, nk_off + 2 * q : nk_off + 2 * q + 2, col_off : col_off + P])
        return w

    # ---- inputs ----
    xt = []
    for j in range(DT):
        t = acts.tile([P, S], F32R, tag=f"xT{j}", name=f"xT{j}")
        nc.sync.dma_start(t[:, 0:SQ], xT_ap[j * P : (j + 1) * P, 0:SQ])
        nc.sync.dma_start(t[:, SQ:S], xT_ap[j * P : (j + 1) * P, SQ:S])
        xt.append(t)
    consts = acts.tile([P, 64], F32, tag="consts", name="consts")
    nc.sync.dma_start(consts[:], consts_ap[:])
    ones_r = acts.tile([P, 1], F32R, tag="ones", name="ones")
    nc.sync.dma_start(ones_r[:], ones_ap[:])
    ones_row = acts.tile([1, P], F32R, tag="ones_row", name="ones_row")
    nc.sync.dma_start(ones_row[:], onesrow_ap[:])
    fold = acts.tile([1, 2 * D], F32R, tag="fold", name="fold")
    nc.sync.dma_start(fold[:], fold_ap[:])

    # ---- Q projection (local 512 query columns), paired output columns ----
    qt = []

    for j0 in range(0, DT, 2):
        wa = wslice(wf, wq_ap, 0, j0 * P)
        wb = wslice(wf, wq_ap, 0, (j0 + 1) * P)
        ps = pp.tile([P, 2, SQ], F32, tag="ps2", name="ps2")
        for k in range(DT):
            nc.tensor.matmul(ps[:, 0, :], wa[:, k, :], xt[k][:, 0:SQ],
                             start=(k == 0), stop=(k == DT - 1))
            nc.tensor.matmul(ps[:, 1, :], wb[:, k, :], xt[k][:, 0:SQ],
                             start=(k == 0), stop=(k == DT - 1))
        for h in range(2):
            q = acts.tile([P, SQ], F32R, tag=f"qT{j0 + h}", name=f"qT{j0 + h}")
            nc.scalar.copy(q[:], ps[:, h, :])
            qt.append(q)

    # ---- K projection (full sequence, both halves share the stationary) ----
    kt = []
    for j in range(DT):
        w = wslice(wf, wk_ap, 0, j * P)
        ps = pp.tile([P, 2, SQ], F32, tag="ps2", name="ps2")
        for k in range(DT):
            nc.tensor.matmul(ps[:, 0, :], w[:, k, :], xt[k][:, 0:SQ],
                             start=(k == 0), stop=(k == DT - 1))
            nc.tensor.matmul(ps[:, 1, :], w[:, k, :], xt[k][:, SQ:S],
                             start=(k == 0), stop=(k == DT - 1))
        kj = acts.tile([P, S], F32R, tag=f"kT{j}", name=f"kT{j}")
        nc.scalar.copy(kj[:].rearrange("p (c q) -> p c q", c=2), ps[:])
        kt.append(kj)

    # ---- V projection, row-major with ones column: vr[st] = [P, H, DH+1] ----
    vr = []
    for st in range(ST):
        t = acts.tile([P, H, DH + 1], BF16, tag=f"vR{st}", name=f"vR{st}")
        nc.scalar.copy(t[:, :, DH : DH + 1], ones_r[:].to_broadcast((P, H, 1)))
        vr.append(t)

    def wv_loads(c):
        wvk = []
        for k in range(DT):
            t = acts.tile([P, SQ], F32R, tag=f"wvc{k}", name=f"wvc{k}")
            nc.sync.dma_start(t[:], wv_ap[k * P : (k + 1) * P, c * SQ : (c + 1) * SQ])
            wvk.append(t)
        return wvk

    def vr_group(c, st, wvk):
        ps = pp.tile([P, 2, SQ], F32, tag="ps2", name="ps2")
        for k in range(DT):
            nc.tensor.matmul(ps[:, 0, :], xt[k][:, st * P : (st + 1) * P],
                             wvk[k][:], start=(k == 0), stop=(k == DT - 1))
        nc.scalar.copy(vr[st][:, c * 8 : (c + 1) * 8, 0:DH],
                       ps[:, 0, :].rearrange("p (h d) -> p h d", d=DH))

    attn = [None] * DT

    def attn_pair(j):
        pv0 = pvp.tile([DH + 1, SQ], F32, tag="pv", name="pv")
        pv1 = pvp.tile([DH + 1, SQ], F32, tag="pv", name="pv")
        for st in range(ST):
            sl = slice(st * P, (st + 1) * P)
            ps = pp.tile([P, 2, SQ], F32, tag="ps2", name="ps2")
            nc.tensor.matmul(ps[:, 0, :], kt[j][0:DH, sl], qt[j][0:DH, :],
                             start=True, stop=True)
            nc.tensor.matmul(ps[:, 1, :], kt[j][DH:P, sl], qt[j][DH:P, :],
                             start=True, stop=True)
            e2 = acts.tile([P, 2, SQ], BF16, tag=f"e{st % 4}", name="e2")
            nc.scalar.activation(e2[:], ps[:], AF.Exp, scale=SCALE)
            nc.tensor.matmul(pv0[:], vr[st][:, 2 * j, :], e2[:, 0, :],
                             start=(st == 0), stop=(st == ST - 1))
            nc.tensor.matmul(pv1[:], vr[st][:, 2 * j + 1, :], e2[:, 1, :],
                             start=(st == 0), stop=(st == ST - 1))
        attn[j] = acts.tile([P, SQ], F32R, tag=f"attnT{j}", name=f"attnT{j}")
        for half, pv in ((0, pv0), (1, pv1)):
            rows = slice(half * DH, half * DH + DH)
            zh = sc2.tile([1, SQ], F32, tag="zh", name="zh")
            nc.vector.tensor_copy(zh[:], pv[DH : DH + 1, :])
            iz = sc2.tile([1, SQ], F32, tag="zh", name="iz")
            nc.vector.reciprocal_approx_fast(iz[:], zh[:])
            bz = sc2.tile([DH, SQ], F32, tag="sb", name="sb")
            nc.gpsimd.partition_broadcast(bz[:], iz[:])
            nc.vector.tensor_mul(attn[j][rows, :], pv[0:DH, :], bz[:])

    # c=0 half of V, then interleave the c=1 half with attention pairs 0..3
    # (pairs 0-3 only touch heads 0-7 = the c=0 section of vr)
    wvk = wv_loads(0)
    for st in range(ST):
        vr_group(0, st, wvk)
    wvk = wv_loads(1)
    for st in range(ST):
        vr_group(1, st, wvk)
        if st % 2 == 1:
            attn_pair(st // 2)
    for j in range(4, DT):
        attn_pair(j)

    def stats_sq(src_j, tag):
        # square on DVE inline with the producer (overlaps PE work); the
        # colsum matmuls are deferred so they never stall the PE FIFO
        sq = acts.tile([P, SQ], F32R, tag=tag, name="sq")
        nc.vector.tensor_mul(sq[:], src_j[:], src_j[:])
        return sq

    def stats_colsums(src, sqs):
        ps_sum = pvp.tile([1, SQ], F32, tag="pv", name="ps_sum")
        ps_sq = pvp.tile([1, SQ], F32, tag="pv", name="ps_sq")
        for j in range(DT):
            nc.tensor.matmul(ps_sum[:], ones_r[:], src[j][:],
                             start=(j == 0), stop=(j == DT - 1))
            nc.tensor.matmul(ps_sq[:], ones_r[:], sqs[j][:],
                             start=(j == 0), stop=(j == DT - 1))
        return ps_sum, ps_sq

    chain_rows = []

    def layernorm(st_ps, src, dst_dtype, dst_tag, g_off, b_off):
        ps_sum, ps_sq = st_ps
        # negD_var = sum^2/D - sumsq  (so var+eps = -negD_var/D + eps)
        s_sb = sc1.tile([1, SQ], F32, tag="s0", name="s_sb")
        nc.vector.tensor_copy(s_sb[:], ps_sum[:])
        m2 = sc1.tile([1, SQ], F32, tag="s1", name="m2")
        nc.vector.tensor_mul(m2[:], s_sb[:], s_sb[:])
        a_t = sc1.tile([1, SQ], F32, tag="s2", name="a_t")
        nc.vector.scalar_tensor_tensor(a_t[:], m2[:], 1.0 / D, ps_sq[:],
                                       op0=mybir.AluOpType.mult,
                                       op1=mybir.AluOpType.subtract)
        eps_t = sc1.tile([1, 1], F32, tag="eps", name="eps")
        nc.vector.memset(eps_t[:], EPS)
        sd = sc1.tile([1, SQ], F32, tag="s1", name="sd")
        nc.scalar.activation(sd[:], a_t[:], AF.Sqrt, bias=eps_t[:],
                             scale=-1.0 / D)
        rstd = sc1.tile([1, SQ], F32, tag="s2", name="rstd")
        nc.vector.reciprocal_approx_fast(rstd[:], sd[:])
        # B = -(sum/D) * rstd
        bneg = sc1.tile([1, SQ], F32, tag="s1", name="bneg")
        nc.vector.scalar_tensor_tensor(bneg[:], s_sb[:], -1.0 / D, rstd[:],
                                       op0=mybir.AluOpType.mult,
                                       op1=mybir.AluOpType.mult)
        chain_rows[:] = [s_sb, sd, rstd, bneg]
        # re-round to f32r (verifier requires f32r producers for PE operands)
        a_r = sc1.tile([1, SQ], F32R, tag="s0", name="a_r")
        nc.vector.tensor_copy(a_r[:], rstd[:])
        b_r = sc1.tile([1, SQ], F32R, tag="s3", name="b_r")
        nc.vector.tensor_copy(b_r[:], bneg[:])
        # broadcast A (rstd) and B along partitions via ones-row outer product
        ab = pp.tile([P, 2, SQ], F32, tag="ps2", name="ab")
        nc.tensor.matmul(ab[:, 0, :], ones_row[:], a_r[:],
                         start=True, stop=True)
        nc.tensor.matmul(ab[:, 1, :], ones_row[:], b_r[:],
                         start=True, stop=True)
        a_sb = sc2.tile([P, SQ], F32, tag="sb", name="a_sb")
        nc.scalar.copy(a_sb[:], ab[:, 0, :])
        b_sb = sc2.tile([P, SQ], F32, tag="zh", name="b_sb")
        nc.scalar.copy(b_sb[:], ab[:, 1, :])
        out = []
        for j in range(DT):
            u = sc2.tile([P, SQ], F32, tag="u", name="u")
            nc.vector.tensor_mul(u[:], src[j][:], a_sb[:])
            nc.vector.tensor_add(u[:], u[:], b_sb[:])
            d = acts.tile([P, SQ], dst_dtype, tag=dst_tag(j), name=f"ln_{dst_tag(j)}")
            nc.scalar.activation(d[:], u[:], AF.Identity,
                                 bias=consts[:, b_off + j : b_off + j + 1],
                                 scale=consts[:, g_off + j : g_off + j + 1])
            out.append(d)
        return out

    # ---- output projection + relu + residual(q_proj) + LN1 ----
    h1, sq1 = [], []
    for j0 in range(0, DT, 2):
        wa = wslice(wf, wo_ap, 0, j0 * P)
        wb = wslice(wf, wo_ap, 0, (j0 + 1) * P)
        ps = pp.tile([P, 2, SQ], F32, tag="ps2", name="ps2")
        for k in range(DT):
            nc.tensor.matmul(ps[:, 0, :], wa[:, k, :], attn[k][:],
                             start=(k == 0), stop=(k == DT - 1))
            nc.tensor.matmul(ps[:, 1, :], wb[:, k, :], attn[k][:],
                             start=(k == 0), stop=(k == DT - 1))
        for h in range(2):
            j = j0 + h
            rel = sc2.tile([P, SQ], F32R, tag="u", name="rel")
            nc.scalar.activation(rel[:], ps[:, h, :], AF.Relu,
                                 bias=consts[:, _C_BO + j : _C_BO + j + 1])
            t = acts.tile([P, SQ], F32R, tag=f"wvc{j}", name=f"h1_{j}")
            nc.vector.tensor_add(t[:], rel[:], qt[j][:])
            h1.append(t)
            sq1.append(stats_sq(t, f"qT{j}"))
    # LN1 stats + chain only; gamma/beta are folded into W1/W2 on the host,
    # so W1 runs directly on h1 and the real ln1 (residual only) is computed
    # off the critical path during W1.
    st1 = stats_colsums(h1, sq1)
    ps_sum1, ps_sq1 = st1
    s_sb = sc1.tile([1, SQ], F32, tag="s0", name="s_sb")
    nc.vector.tensor_copy(s_sb[:], ps_sum1[:])
    m2 = sc1.tile([1, SQ], F32, tag="s1", name="m2")
    nc.vector.tensor_mul(m2[:], s_sb[:], s_sb[:])
    a_t = sc1.tile([1, SQ], F32, tag="s2", name="a_t")
    nc.vector.scalar_tensor_tensor(a_t[:], m2[:], 1.0 / D, ps_sq1[:],
                                   op0=mybir.AluOpType.mult,
                                   op1=mybir.AluOpType.subtract)
    eps_t = sc1.tile([1, 1], F32, tag="eps", name="eps")
    nc.vector.memset(eps_t[:], EPS)
    sd1 = sc1.tile([1, SQ], F32, tag="s1", name="sd1")
    nc.scalar.activation(sd1[:], a_t[:], AF.Sqrt, bias=eps_t[:], scale=-1.0 / D)
    rstd1 = sc1.tile([1, SQ], F32, tag="s2", name="rstd1")
    nc.vector.reciprocal_approx_fast(rstd1[:], sd1[:])
    bneg1 = sc1.tile([1, SQ], F32, tag="s3", name="bneg1")
    nc.vector.scalar_tensor_tensor(bneg1[:], s_sb[:], -1.0 / D, rstd1[:],
                                   op0=mybir.AluOpType.mult,
                                   op1=mybir.AluOpType.mult)
    # f32r rows for the W2-stage rank-1 fold matmuls
    negmu_r = sc1.tile([1, SQ], F32R, tag="s4", name="negmu_r")
    nc.vector.tensor_scalar_mul(negmu_r[:], s_sb[:], -1.0 / D)
    sd_r = sc1.tile([1, SQ], F32R, tag="s5", name="sd_r")
    nc.vector.tensor_copy(sd_r[:], sd1[:])
    # SBUF broadcasts of A=rstd and B=-mu*rstd (gpsimd; off critical path)
    abc_sb = sc2.tile([P, SQ], F32, tag="sb", name="abc_sb")
    nc.gpsimd.partition_broadcast(abc_sb[:], rstd1[:])
    bbc_sb = sc2.tile([P, SQ], F32, tag="zh", name="bbc_sb")
    nc.gpsimd.partition_broadcast(bbc_sb[:], bneg1[:])

    # ---- FFN (reference: ff = relu((out @ W1 + b1) @ W2 + b2) + out) ----
    # W1 here is gamma1-scaled on the host; hid = G = (g1*W1)^T h1.
    hid = [None] * DT

    def w1slice(f):
        if f < 8:
            slot = 4 + f if f < 4 else f - 4
            w = acts.tile([P, 8, P], F32R, tag=f"xT{slot}", name="w1pre")
            srcw = w1_ap.rearrange("(k p) n -> p k n", p=P)
            for q in range(4):
                nc.sync.dma_start(
                    w[:, 2 * q : 2 * q + 2, :],
                    srcw[:, 2 * q : 2 * q + 2, f * P : (f + 1) * P])
            return w
        return wslice(wf, w1_ap, 0, f * P)

    for f0 in range(0, FT, 2):
        wa = w1slice(f0)
        wb = w1slice(f0 + 1)
        ps = pp.tile([P, 2, SQ], F32, tag="ps2", name="ps2")
        for k in range(DT):
            nc.tensor.matmul(ps[:, 0, :], wa[:, k, :], h1[k][:],
                             start=(k == 0), stop=(k == DT - 1))
            nc.tensor.matmul(ps[:, 1, :], wb[:, k, :], h1[k][:],
                             start=(k == 0), stop=(k == DT - 1))
        for h in range(2):
            f = f0 + h
            m, half = f % DT, (f // DT) * SQ
            if hid[m] is None:
                hid[m] = acts.tile([P, S], F32R, tag=f"kT{m}", name=f"hid{m}")
            nc.scalar.copy(hid[m][:, half : half + SQ], ps[:, h, :])

    # real ln1 for the residual only (during W1; reads SBUF broadcasts)
    ln1 = []
    for j in range(DT):
        u = sc2.tile([P, SQ], F32, tag="u", name="u")
        nc.vector.tensor_mul(u[:], h1[j][:], abc_sb[:])
        nc.vector.tensor_add(u[:], u[:], bbc_sb[:])
        d = acts.tile([P, SQ], F32R, tag=f"attnT{j}", name=f"ln1_{j}")
        nc.scalar.activation(d[:], u[:], AF.Identity,
                             bias=consts[:, _C_BE1 + j : _C_BE1 + j + 1],
                             scale=consts[:, _C_G1 + j : _C_G1 + j + 1])
        ln1.append(d)

    f2, sq2 = [], []

    def w2slice(j, half):
        w = acts.tile([P, 8, P], F32R, tag=f"xT{(2 * j + half) % 8}", name="w2t")
        srcw = w2_ap.rearrange("(k p) n -> p k n", p=P)
        for q in range(4):
            nc.sync.dma_start(
                w[:, 2 * q : 2 * q + 2, :],
                srcw[:, 8 * half + 2 * q : 8 * half + 2 * q + 2,
                     j * P : (j + 1) * P])
        return w

    for j in range(DT):
        wa = w2slice(j, 0)
        wb = w2slice(j, 1)
        ps = pp.tile([P, 2, SQ], F32, tag="ps2", name="ps2")
        for f in range(FT):
            w = wa if f < 8 else wb
            m, half = f % DT, (f // DT) * SQ
            nc.tensor.matmul(ps[:, 0, :], w[:, f % 8, :],
                             hid[m][:, half : half + SQ],
                             start=(f == 0), stop=False)
        # rank-1 corrections: + (-mu) x w2g1[d]  + sd x c2[d]
        nc.tensor.matmul(ps[:, 0, :], fold[0:1, j * P : (j + 1) * P],
                         negmu_r[:], start=False, stop=False)
        nc.tensor.matmul(ps[:, 0, :], fold[0:1, D + j * P : D + (j + 1) * P],
                         sd_r[:], start=False, stop=True)
        # ff_pre = A * psum ; relu(A*x) = A*relu(x) since A=rstd>0
        rel = sc2.tile([P, SQ], F32R, tag="u", name="rel2")
        nc.vector.scalar_tensor_tensor(rel[:], ps[:, 0, :], 0.0, abc_sb[:],
                                       op0=mybir.AluOpType.max,
                                       op1=mybir.AluOpType.mult)
        t = acts.tile([P, SQ], F32R, tag=f"qT{j}", name=f"f2_{j}")
        nc.vector.tensor_add(t[:], rel[:], ln1[j][:])
        f2.append(t)
        sq2.append(stats_sq(t, f"wvc{j}"))
    yt = layernorm(stats_colsums(f2, sq2), f2, F32,
                   lambda j: f"attnT{j}", _C_G2, _C_BE2)

    for j in range(DT):
        nc.sync.dma_start(yT_ap[j * P : (j + 1) * P, 0 : SQ // 2],
                          yt[j][:, 0 : SQ // 2])
        nc.sync.dma_start(yT_ap[j * P : (j + 1) * P, SQ // 2 : SQ],
                          yt[j][:, SQ // 2 : SQ])


def build():
    nc = bacc.Bacc("TRN2", target_bir_lowering=False, debug=False,
                   num_devices=NCORES)
    xT_ap = nc.dram_tensor("xT", [D, S], F32R, kind="ExternalInput").ap()
    wq_ap = nc.dram_tensor("Wq", [D, D], F32R, kind="ExternalInput").ap()
    wk_ap = nc.dram_tensor("Wk", [D, D], F32R, kind="ExternalInput").ap()
    wv_ap = nc.dram_tensor("Wv", [D, D], F32R, kind="ExternalInput").ap()
    wo_ap = nc.dram_tensor("Wo", [D, D], F32R, kind="ExternalInput").ap()
    w1_ap = nc.dram_tensor("W1", [D, FF], F32R, kind="ExternalInput").ap()
    w2_ap = nc.dram_tensor("W2", [FF, D], F32R, kind="ExternalInput").ap()
    consts_ap = nc.dram_tensor("consts", [P, 64], F32, kind="ExternalInput").ap()
    ones_ap = nc.dram_tensor("ones", [P, 1], F32R, kind="ExternalInput").ap()
    onesrow_ap = nc.dram_tensor("ones_row", [1, P], F32R, kind="ExternalInput").ap()
    fold_ap = nc.dram_tensor("fold", [1, 2 * D], F32R, kind="ExternalInput").ap()
    yT_ap = nc.dram_tensor("yT", [D, SQ], F32, kind="ExternalOutput").ap()
    aps = (xT_ap, wq_ap, wk_ap, wv_ap, wo_ap, w1_ap, w2_ap, consts_ap, ones_ap, onesrow_ap, fold_ap, yT_ap)
    from contextlib import ExitStack
    with tile.TileContext(nc) as tc, ExitStack() as ctx:
        _emit(ctx, tc, aps)
    nc.compile()
    return nc


_cached_nc = None


def _get_nc():
    global _cached_nc
    if _cached_nc is None:
        _cached_nc = build()
    return _cached_nc


def _prep_in_maps(x, Wq, Wk, Wv, Wo, bo, ln1_g, ln1_b, W1, b1, W2, b2,
                  ln2_g, ln2_b):
    f = np.float32
    consts = np.zeros((P, 64), f)
    consts[:, _C_BO:_C_BO + 8] = np.asarray(bo, f).reshape(8, P).T
    consts[:, _C_B1:_C_B1 + 16] = np.asarray(b1, f).reshape(16, P).T
    consts[:, _C_B2:_C_B2 + 8] = np.asarray(b2, f).reshape(8, P).T
    consts[:, _C_G1:_C_G1 + 8] = np.asarray(ln1_g, f).reshape(8, P).T
    consts[:, _C_BE1:_C_BE1 + 8] = np.asarray(ln1_b, f).reshape(8, P).T
    consts[:, _C_G2:_C_G2 + 8] = np.asarray(ln2_g, f).reshape(8, P).T
    consts[:, _C_BE2:_C_BE2 + 8] = np.asarray(ln2_b, f).reshape(8, P).T
    ones = np.ones((P, 1), f)
    ones_row = np.ones((1, P), f)
    W1f = np.asarray(W1, np.float64)
    W2f = np.asarray(W2, np.float64)
    g1v = np.asarray(ln1_g, np.float64)
    b1v = np.asarray(ln1_b, np.float64)
    g1 = (g1v[:, None] * W1f).sum(axis=0)            # [FF]
    c1 = np.asarray(b1, np.float64) + (b1v[:, None] * W1f).sum(axis=0)
    w2g1 = g1 @ W2f                                   # [D]
    c2 = np.asarray(b2, np.float64) + c1 @ W2f        # [D]
    fold = np.concatenate([w2g1, c2]).astype(f)[None, :]
    W1g = (g1v[:, None] * W1f).astype(f)
    shared = {
        "Wq": np.ascontiguousarray(Wq, f), "Wk": np.ascontiguousarray(Wk, f),
        "Wv": np.ascontiguousarray(Wv, f), "Wo": np.ascontiguousarray(Wo, f),
        "W1": np.ascontiguousarray(W1g, f), "W2": np.ascontiguousarray(W2, f),
        "consts": consts, "ones": ones, "ones_row": ones_row, "fold": fold,
    }
    xt = np.ascontiguousarray(np.asarray(x, f).transpose(0, 2, 1))  # [B, D, S]
    in_maps = []
    for core in range(NCORES):
        b, off = core // 2, (core % 2) * SQ
        if off == 0:
            xrot = xt[b]
        else:
            # rotate so this core's query rows are columns 0:SQ; key order is
            # irrelevant (softmax sums over all keys)
            xrot = np.ascontiguousarray(
                np.concatenate([xt[b][:, off:], xt[b][:, :off]], axis=1))
        in_maps.append(dict(shared, xT=xrot))
    return in_maps


def run(inputs, trace=False, tmpdir=None):
    """Run the kernel on 8 cores. Returns (y, BassKernelResults)."""
    nc = _get_nc()
    in_maps = _prep_in_maps(
        inputs["x"], inputs["Wq"], inputs["Wk"], inputs["Wv"], inputs["Wo"],
        inputs["bo"], inputs["ln1_g"], inputs["ln1_b"], inputs["W1"],
        inputs["b1"], inputs["W2"], inputs["b2"], inputs["ln2_g"],
        inputs["ln2_b"])
    try:
        res = bass_utils.run_bass_kernel_spmd(nc, in_maps, list(range(NCORES)),
                                              trace=trace, tmpdir=tmpdir)
    except Exception:
        # transient NRT wedge right after NEFF load; retry once on a clean run
        import time as _time
        _time.sleep(2.0)
        res = bass_utils.run_bass_kernel_spmd(nc, in_maps, list(range(NCORES)),
                                              trace=trace, tmpdir=tmpdir)
    y = np.empty((B, S, D), np.float32)
    for core in range(NCORES):
        b, off = core // 2, (core % 2) * SQ
        y[b, off:off + SQ, :] = res.results[core]["yT"].T
    return y, res


def kernel(x, mask, Wq, Wk, Wv, Wo, bo, ln1_g, ln1_b, W1, b1, W2, b2,
           ln2_g, ln2_b):
    # mask is all-ones per the problem spec (fill: ones) -> identity in the
    # reference's jnp.where; accepted but unused.
    y, _ = run(dict(x=x, Wq=Wq, Wk=Wk, Wv=Wv, Wo=Wo, bo=bo, ln1_g=ln1_g,
                    ln1_b=ln1_b, W1=W1, b1=b1, W2=W2, b2=b2, ln2_g=ln2_g,
                    ln2_b=ln2_b))
    return y



# revision 4
# speedup vs baseline: 1.1804x; 1.1804x over previous
"""Trainium2 Bass kernel for nn_EncoderLayer (B=4, S=1024, D=1024, H=16, FF=2048).

Sharding: 8 cores = 4 batches x 2 sequence-halves (as the baseline), each core
redundantly computes K/V for its whole batch and runs the layer for its own
512 query rows. Odd cores see the sequence rotated by 512.

v2: mixed-precision PE pipeline.
  - K/V projections, QK^T scores, exp, attn*V and the output projection run in
    fp8e4 (TRN E4M3), with MatmulPerfMode.DoubleRow (two 128-row contraction
    tiles per instruction = 2x PE throughput) everywhere the contraction is a
    multiple of 256. Weights are pre-scaled by 32 on the host so they sit in
    e4m3's normal range; the 1/32 is folded into the PSUM->SBUF copies.
  - Q projection and both FFN matmuls stay bf16: their error lands directly on
    the residual stream (q_proj residual, FFN output) where fp8 would blow the
    2e-2 budget (measured 5e-2 all-fp8 vs 3.2e-3 with these three in bf16).
  - The softmax exp (the scalar engine's dominant cost, ~57us) is started as
    early as possible: scores for head-pair j are emitted right after the K
    projection of feature tile j, and attn*V for pairs 0-3 only needs the
    first half of V, so V(c=0) is hoisted before the K loop.

Layernorm plumbing (feature-major activations, stats via ones-column matmuls,
gamma/beta folded into W1/rank-1 W2 corrections) is inherited from v1.
"""

import sys
import types

import numpy as np
import ml_dtypes


def _shim_axon_hooks():
    try:
        import antenv.axon_hooks  # noqa: F401
    except Exception:
        mod = types.ModuleType("antenv.axon_hooks")
        mod.get_axon_ntff_profile_hook = lambda: None
        mod.set_axon_ntff_profile_hook = lambda h: None
        sys.modules["antenv.axon_hooks"] = mod


_shim_axon_hooks()

from concourse import bacc, mybir, tile  # noqa: E402
from concourse import bass_utils  # noqa: E402

F32 = mybir.dt.float32
F32R = mybir.dt.float32r
BF16 = mybir.dt.bfloat16
FP8 = mybir.dt.float8e4
AF = mybir.ActivationFunctionType
DR = mybir.MatmulPerfMode.DoubleRow

B, S, D, H, DH, FF = 4, 1024, 1024, 16, 64, 2048
SQ = 512          # query rows per core
P = 128
DT = D // P       # 8 d_model tiles
FT = FF // P      # 16 ffn tiles
ST = S // P       # 8 key tiles
NCORES = 8
EPS = 1e-6
SCALE = 1.0 / 32.0  # 1/sqrt(D_MODEL)
ALPHA = 32.0        # fp8 weight pre-scale (host side)

# consts layout (one [128, 64] f32 array, column ranges):
_C_BO = 0    # 8 cols: bo per d-tile
_C_B1 = 8    # 16 cols: b1 per f-tile (folded into `fold`, kept for layout)
_C_B2 = 24   # 8 cols
_C_G1 = 32   # 8 cols
_C_BE1 = 40  # 8 cols
_C_G2 = 48   # 8 cols
_C_BE2 = 56  # 8 cols


def _emit(ctx, tc, aps):
    nc = tc.nc
    (xT_ap, wq_ap, wk_ap, wv_ap, wo_ap, w1_ap, w2_ap, consts_ap, ones_ap,
     onesrow_ap, fold_ap, yT_ap) = aps

    acts = ctx.enter_context(tc.tile_pool(name="acts", bufs=1))
    wts = ctx.enter_context(tc.tile_pool(name="wts", bufs=2))
    e2p = ctx.enter_context(tc.tile_pool(name="e2p", bufs=6))
    sc2 = ctx.enter_context(tc.tile_pool(name="sc2", bufs=2))
    sc1 = ctx.enter_context(tc.tile_pool(name="sc1", bufs=1))
    pp = ctx.enter_context(tc.tile_pool(name="pp", bufs=2, space="PSUM"))
    pvp = ctx.enter_context(tc.tile_pool(name="pvp", bufs=4, space="PSUM"))

    # ---- inputs ----
    xtb = acts.tile([P, DT, S], BF16, tag="xtb", name="xtb")
    for q in range(4):
        nc.sync.dma_start(xtb[:, 2 * q: 2 * q + 2, :],
                          xT_ap[:, 2 * q: 2 * q + 2, :])
    consts = acts.tile([P, 64], F32, tag="consts", name="consts")
    nc.sync.dma_start(consts[:], consts_ap[:])
    ones_r = acts.tile([P, 1], F32R, tag="ones", name="ones")
    nc.sync.dma_start(ones_r[:], ones_ap[:])
    ones_row = acts.tile([1, P], F32R, tag="ones_row", name="ones_row")
    nc.sync.dma_start(ones_row[:], onesrow_ap[:])
    fold = acts.tile([1, 2 * D], BF16, tag="fold", name="fold")
    nc.sync.dma_start(fold[:], fold_ap[:])

    def wload(tag, src_ap, shape, dtype, nsplit=2):
        w = wts.tile(shape, dtype, tag=tag, name=tag)
        n1 = shape[1]
        step = n1 // nsplit
        for q in range(nsplit):
            nc.sync.dma_start(w[:, q * step:(q + 1) * step, :],
                              src_ap[:, q * step:(q + 1) * step, :])
        return w

    # ---- Q projection (bf16, local 512 query columns), paired outputs ----
    q_res, q8 = [], acts.tile([P, DT, SQ], FP8, tag="q8", name="q8")
    for j0 in range(0, DT, 2):
        wa = wload("wqa", wq_ap[:, j0], [P, DT, P], BF16)
        wb = wload("wqb", wq_ap[:, j0 + 1], [P, DT, P], BF16)
        ps = pp.tile([P, 2, SQ], F32, tag="ps2", name="ps2")
        for k in range(DT):
            nc.tensor.matmul(ps[:, 0, :], wa[:, k, :], xtb[:, k, 0:SQ],
                             start=(k == 0), stop=(k == DT - 1))
            nc.tensor.matmul(ps[:, 1, :], wb[:, k, :], xtb[:, k, 0:SQ],
                             start=(k == 0), stop=(k == DT - 1))
        for h in range(2):
            j = j0 + h
            qr = acts.tile([P, SQ], F32R, tag=f"qres{j}", name=f"qres{j}")
            nc.scalar.copy(qr[:], ps[:, h, :])
            q_res.append(qr)
            nc.vector.tensor_copy(q8[:, j, :], ps[:, h, :])

    # xt8 conversion for K/V (vector engine, overlaps Q's PE work)
    xt8 = acts.tile([P, DT, S], FP8, tag="xt8", name="xt8")
    for q in range(4):
        nc.vector.tensor_copy(xt8[:, 2 * q: 2 * q + 2, :],
                              xtb[:, 2 * q: 2 * q + 2, :])

    # ---- V projection (fp8 DoubleRow, row-major with ones column) ----
    vr8 = acts.tile([P, ST, H, DH + 1], FP8, tag="vr8", name="vr8")
    nc.vector.memset(vr8[:, :, :, DH:DH + 1], 1.0)

    def v_half(c, wv):
        for st in range(ST):
            ps = pp.tile([P, 2, SQ], F32, tag="ps2", name="ps2")
            for k in range(0, DT, 2):
                nc.tensor.matmul(ps[:, 0, :],
                                 xt8[:, k:k + 2, st * P:(st + 1) * P],
                                 wv[:, k:k + 2, :],
                                 start=(k == 0), stop=(k == DT - 2),
                                 perf_mode=DR)
            nc.vector.tensor_scalar_mul(
                vr8[:, st, c * 8:(c + 1) * 8, 0:DH],
                ps[:, 0, :].rearrange("p (h d) -> p h d", d=DH), 1.0 / ALPHA)

    attn8 = acts.tile([P, DT, SQ], FP8, tag="attn8", name="attn8")

    def attn_pair(j):
        """scores + exp for head pair (2j, 2j+1), then attn*V and normalize."""
        pv0 = pvp.tile([DH + 1, SQ], F32, tag="pv", name="pv")
        pv1 = pvp.tile([DH + 1, SQ], F32, tag="pv", name="pv")
        k8j = k8[j]
        for st2 in range(ST // 2):
            e2 = e2p.tile([P, 2, 2, SQ], FP8, tag="e2", name="e2")
            for sti in range(2):
                st = 2 * st2 + sti
                sl = slice(st * P, (st + 1) * P)
                ps = pp.tile([P, 2, SQ], F32, tag="ps2", name="ps2")
                nc.tensor.matmul(ps[:, 0, :], k8j[0:DH, sl], q8[0:DH, j, :],
                                 start=True, stop=True)
                nc.tensor.matmul(ps[:, 1, :], k8j[DH:P, sl], q8[DH:P, j, :],
                                 start=True, stop=True)
                nc.scalar.activation(e2[:, sti, :, :], ps[:], AF.Exp,
                                     scale=SCALE)
            nc.tensor.matmul(pv0[:], vr8[:, 2 * st2:2 * st2 + 2, 2 * j, :],
                             e2[:, :, 0, :], start=(st2 == 0),
                             stop=(st2 == ST // 2 - 1), perf_mode=DR)
            nc.tensor.matmul(pv1[:], vr8[:, 2 * st2:2 * st2 + 2, 2 * j + 1, :],
                             e2[:, :, 1, :], start=(st2 == 0),
                             stop=(st2 == ST // 2 - 1), perf_mode=DR)
        for half, pv in ((0, pv0), (1, pv1)):
            rows = slice(half * DH, half * DH + DH)
            zh = sc2.tile([1, SQ], F32, tag="zh", name="zh")
            nc.vector.tensor_copy(zh[:], pv[DH:DH + 1, :])
            iz = sc2.tile([1, SQ], F32, tag="zh", name="iz")
            nc.vector.reciprocal_approx_fast(iz[:], zh[:])
            bz = sc2.tile([DH, SQ], F32, tag="sb", name="sb")
            nc.gpsimd.partition_broadcast(bz[:], iz[:])
            nc.vector.tensor_mul(attn8[rows, j, :], pv[0:DH, :], bz[:])

    # ---- K projection (fp8 DoubleRow, full sequence) + early attention ----
    # V(c=0) first so attn pairs 0-3 (heads 0-7) can run inside the K loop;
    # this gets the scalar engine's exp stream (the critical ~57us) started
    # as early as possible.
    k8 = []
    wv0 = wload("wv", wv_ap[:, 0], [P, DT, SQ], FP8, nsplit=4)
    v_half(0, wv0)
    for j in range(DT):
        w = wload("wka" if j % 2 == 0 else "wkb", wk_ap[:, j], [P, DT, P], FP8)
        ps = pp.tile([P, 2, SQ], F32, tag="ps2", name="ps2")
        for k in range(0, DT, 2):
            nc.tensor.matmul(ps[:, 0, :], w[:, k:k + 2, :],
                             xt8[:, k:k + 2, 0:SQ],
                             start=(k == 0), stop=(k == DT - 2), perf_mode=DR)
            nc.tensor.matmul(ps[:, 1, :], w[:, k:k + 2, :],
                             xt8[:, k:k + 2, SQ:S],
                             start=(k == 0), stop=(k == DT - 2), perf_mode=DR)
        kj = acts.tile([P, S], FP8, tag=f"k8{j}", name=f"k8{j}")
        nc.vector.tensor_scalar_mul(
            kj[:].rearrange("p (c q) -> p c q", c=2), ps[:], 1.0 / ALPHA)
        k8.append(kj)
        if j == 3:
            wv1 = wload("wv", wv_ap[:, 1], [P, DT, SQ], FP8, nsplit=4)
            v_half(1, wv1)
        if j >= 1:
            attn_pair(j - 1)
    attn_pair(DT - 1)

    def stats_sq(src_j, tag):
        sq = acts.tile([P, SQ], F32R, tag=tag, name="sq")
        nc.vector.tensor_mul(sq[:], src_j[:], src_j[:])
        return sq

    def stats_colsums(src, sqs):
        ps_sum = pvp.tile([1, SQ], F32, tag="pv", name="ps_sum")
        ps_sq = pvp.tile([1, SQ], F32, tag="pv", name="ps_sq")
        for j in range(DT):
            nc.tensor.matmul(ps_sum[:], ones_r[:], src[j][:],
                             start=(j == 0), stop=(j == DT - 1))
            nc.tensor.matmul(ps_sq[:], ones_r[:], sqs[j][:],
                             start=(j == 0), stop=(j == DT - 1))
        return ps_sum, ps_sq

    def layernorm(st_ps, src, dst_dtype, dst_tag, g_off, b_off):
        ps_sum, ps_sq = st_ps
        s_sb = sc1.tile([1, SQ], F32, tag="s0", name="s_sb")
        nc.vector.tensor_copy(s_sb[:], ps_sum[:])
        m2 = sc1.tile([1, SQ], F32, tag="s1", name="m2")
        nc.vector.tensor_mul(m2[:], s_sb[:], s_sb[:])
        a_t = sc1.tile([1, SQ], F32, tag="s2", name="a_t")
        nc.vector.scalar_tensor_tensor(a_t[:], m2[:], 1.0 / D, ps_sq[:],
                                       op0=mybir.AluOpType.mult,
                                       op1=mybir.AluOpType.subtract)
        eps_t = sc1.tile([1, 1], F32, tag="eps", name="eps")
        nc.vector.memset(eps_t[:], EPS)
        sd = sc1.tile([1, SQ], F32, tag="s1", name="sd")
        nc.scalar.activation(sd[:], a_t[:], AF.Sqrt, bias=eps_t[:],
                             scale=-1.0 / D)
        rstd = sc1.tile([1, SQ], F32, tag="s2", name="rstd")
        nc.vector.reciprocal_approx_fast(rstd[:], sd[:])
        bneg = sc1.tile([1, SQ], F32, tag="s1", name="bneg")
        nc.vector.scalar_tensor_tensor(bneg[:], s_sb[:], -1.0 / D, rstd[:],
                                       op0=mybir.AluOpType.mult,
                                       op1=mybir.AluOpType.mult)
        a_r = sc1.tile([1, SQ], F32R, tag="s0", name="a_r")
        nc.vector.tensor_copy(a_r[:], rstd[:])
        b_r = sc1.tile([1, SQ], F32R, tag="s3", name="b_r")
        nc.vector.tensor_copy(b_r[:], bneg[:])
        ab = pp.tile([P, 2, SQ], F32, tag="ps2", name="ab")
        nc.tensor.matmul(ab[:, 0, :], ones_row[:], a_r[:], start=True, stop=True)
        nc.tensor.matmul(ab[:, 1, :], ones_row[:], b_r[:], start=True, stop=True)
        a_sb = sc2.tile([P, SQ], F32, tag="sb", name="a_sb")
        nc.scalar.copy(a_sb[:], ab[:, 0, :])
        b_sb = sc2.tile([P, SQ], F32, tag="zh", name="b_sb")
        nc.scalar.copy(b_sb[:], ab[:, 1, :])
        out = []
        for j in range(DT):
            u = sc2.tile([P, SQ], F32, tag="u", name="u")
            nc.vector.tensor_mul(u[:], src[j][:], a_sb[:])
            nc.vector.tensor_add(u[:], u[:], b_sb[:])
            d = acts.tile([P, SQ], dst_dtype, tag=dst_tag(j), name=f"ln_{j}")
            nc.scalar.activation(d[:], u[:], AF.Identity,
                                 bias=consts[:, b_off + j:b_off + j + 1],
                                 scale=consts[:, g_off + j:g_off + j + 1])
            out.append(d)
        return out

    # ---- output projection (fp8 DoubleRow) + relu + residual + LN1 stats ----
    h1, sq1 = [], []
    h18 = acts.tile([P, DT, SQ], BF16, tag="h18", name="h18")
    for j0 in range(0, DT, 2):
        wa = wload("woa", wo_ap[:, j0], [P, DT, P], FP8)
        wb = wload("wob", wo_ap[:, j0 + 1], [P, DT, P], FP8)
        ps = pp.tile([P, 2, SQ], F32, tag="ps2", name="ps2")
        for k in range(0, DT, 2):
            nc.tensor.matmul(ps[:, 0, :], wa[:, k:k + 2, :],
                             attn8[:, k:k + 2, :],
                             start=(k == 0), stop=(k == DT - 2), perf_mode=DR)
            nc.tensor.matmul(ps[:, 1, :], wb[:, k:k + 2, :],
                             attn8[:, k:k + 2, :],
                             start=(k == 0), stop=(k == DT - 2), perf_mode=DR)
        for h in range(2):
            j = j0 + h
            rel = sc2.tile([P, SQ], F32R, tag="u", name="rel")
            nc.scalar.activation(rel[:], ps[:, h, :], AF.Relu,
                                 bias=consts[:, _C_BO + j:_C_BO + j + 1],
                                 scale=1.0 / ALPHA)
            t = acts.tile([P, SQ], F32R, tag=f"h1{j}", name=f"h1_{j}")
            nc.vector.tensor_add(t[:], rel[:], q_res[j][:])
            h1.append(t)
            nc.gpsimd.tensor_copy(h18[:, j, :], t[:])
            sq1.append(stats_sq(t, f"sq1{j}"))

    # LN1 stats + chain (gamma/beta folded into W1 / rank-1 W2 fold)
    ps_sum1, ps_sq1 = stats_colsums(h1, sq1)
    s_sb = sc1.tile([1, SQ], F32, tag="s0", name="s_sb")
    nc.vector.tensor_copy(s_sb[:], ps_sum1[:])
    m2 = sc1.tile([1, SQ], F32, tag="s1", name="m2")
    nc.vector.tensor_mul(m2[:], s_sb[:], s_sb[:])
    a_t = sc1.tile([1, SQ], F32, tag="s2", name="a_t")
    nc.vector.scalar_tensor_tensor(a_t[:], m2[:], 1.0 / D, ps_sq1[:],
                                   op0=mybir.AluOpType.mult,
                                   op1=mybir.AluOpType.subtract)
    eps_t = sc1.tile([1, 1], F32, tag="eps", name="eps")
    nc.vector.memset(eps_t[:], EPS)
    sd1 = sc1.tile([1, SQ], F32, tag="s1", name="sd1")
    nc.scalar.activation(sd1[:], a_t[:], AF.Sqrt, bias=eps_t[:], scale=-1.0 / D)
    rstd1 = sc1.tile([1, SQ], F32, tag="s2", name="rstd1")
    nc.vector.reciprocal_approx_fast(rstd1[:], sd1[:])
    bneg1 = sc1.tile([1, SQ], F32, tag="s3", name="bneg1")
    nc.vector.scalar_tensor_tensor(bneg1[:], s_sb[:], -1.0 / D, rstd1[:],
                                   op0=mybir.AluOpType.mult,
                                   op1=mybir.AluOpType.mult)
    negmu_r = sc1.tile([1, SQ], BF16, tag="s4", name="negmu_r")
    nc.vector.tensor_scalar_mul(negmu_r[:], s_sb[:], -1.0 / D)
    sd_r = sc1.tile([1, SQ], BF16, tag="s5", name="sd_r")
    nc.vector.tensor_copy(sd_r[:], sd1[:])
    abc_sb = sc2.tile([P, SQ], F32, tag="sb", name="abc_sb")
    nc.gpsimd.partition_broadcast(abc_sb[:], rstd1[:])
    bbc_sb = sc2.tile([P, SQ], F32, tag="zh", name="bbc_sb")
    nc.gpsimd.partition_broadcast(bbc_sb[:], bneg1[:])

    # ---- FFN1 (bf16; W1 is gamma1-scaled on the host) ----
    hid_a = acts.tile([P, DT, SQ], BF16, tag="xtb", name="hid_a")
    hid_b = acts.tile([P, DT, SQ], BF16, tag="hid_b", name="hid_b")
    for f0 in range(0, FT, 2):
        wa = wload("w1a", w1_ap[:, f0], [P, DT, P], BF16)
        wb = wload("w1b", w1_ap[:, f0 + 1], [P, DT, P], BF16)
        ps = pp.tile([P, 2, SQ], F32, tag="ps2", name="ps2")
        for k in range(DT):
            nc.tensor.matmul(ps[:, 0, :], wa[:, k, :], h18[:, k, :],
                             start=(k == 0), stop=(k == DT - 1))
            nc.tensor.matmul(ps[:, 1, :], wb[:, k, :], h18[:, k, :],
                             start=(k == 0), stop=(k == DT - 1))
        hid = hid_a if f0 < DT else hid_b
        nc.scalar.copy(hid[:, f0 % DT:f0 % DT + 2, :], ps[:])

    # real ln1 for the residual (during FFN1; reads SBUF broadcasts)
    ln1 = []
    for j in range(DT):
        u = sc2.tile([P, SQ], F32, tag="u", name="u")
        nc.vector.tensor_mul(u[:], h1[j][:], abc_sb[:])
        nc.vector.tensor_add(u[:], u[:], bbc_sb[:])
        d = acts.tile([P, SQ], F32R, tag=f"ln1{j}", name=f"ln1_{j}")
        nc.scalar.activation(d[:], u[:], AF.Identity,
                             bias=consts[:, _C_BE1 + j:_C_BE1 + j + 1],
                             scale=consts[:, _C_G1 + j:_C_G1 + j + 1])
        ln1.append(d)

    # ---- FFN2 (bf16) + rank-1 LN1 fold + relu + residual + LN2 stats ----
    f2, sq2 = [], []
    for j in range(DT):
        w = wload("w2", w2_ap[:, j], [P, FT, P], BF16, nsplit=4)
        ps = pp.tile([P, 2, SQ], F32, tag="ps2", name="ps2")
        for f in range(FT):
            hid = hid_a if f < DT else hid_b
            nc.tensor.matmul(ps[:, 0, :], w[:, f, :], hid[:, f % DT, :],
                             start=(f == 0), stop=False)
        nc.tensor.matmul(ps[:, 0, :], fold[0:1, j * P:(j + 1) * P],
                         negmu_r[:], start=False, stop=False)
        nc.tensor.matmul(ps[:, 0, :], fold[0:1, D + j * P:D + (j + 1) * P],
                         sd_r[:], start=False, stop=True)
        rel = sc2.tile([P, SQ], F32R, tag="u", name="rel2")
        nc.vector.scalar_tensor_tensor(rel[:], ps[:, 0, :], 0.0, abc_sb[:],
                                       op0=mybir.AluOpType.max,
                                       op1=mybir.AluOpType.mult)
        t = acts.tile([P, SQ], F32R, tag=f"h1{j}", name=f"f2_{j}")
        nc.vector.tensor_add(t[:], rel[:], ln1[j][:])
        f2.append(t)
        sq2.append(stats_sq(t, f"sq1{j}"))
    yt = layernorm(stats_colsums(f2, sq2), f2, F32,
                   lambda j: f"qres{j}", _C_G2, _C_BE2)

    for j in range(DT):
        nc.sync.dma_start(yT_ap[j * P:(j + 1) * P, 0:SQ // 2],
                          yt[j][:, 0:SQ // 2])
        nc.sync.dma_start(yT_ap[j * P:(j + 1) * P, SQ // 2:SQ],
                          yt[j][:, SQ // 2:SQ])


def build():
    nc = bacc.Bacc("TRN2", target_bir_lowering=False, debug=False,
                   num_devices=NCORES)
    xT_ap = nc.dram_tensor("xT", [P, DT, S], BF16, kind="ExternalInput").ap()
    wq_ap = nc.dram_tensor("Wq", [P, DT, DT, P], BF16, kind="ExternalInput").ap()
    wk_ap = nc.dram_tensor("Wk", [P, DT, DT, P], FP8, kind="ExternalInput").ap()
    wv_ap = nc.dram_tensor("Wv", [P, 2, DT, SQ], FP8, kind="ExternalInput").ap()
    wo_ap = nc.dram_tensor("Wo", [P, DT, DT, P], FP8, kind="ExternalInput").ap()
    w1_ap = nc.dram_tensor("W1", [P, FT, DT, P], BF16, kind="ExternalInput").ap()
    w2_ap = nc.dram_tensor("W2", [P, DT, FT, P], BF16, kind="ExternalInput").ap()
    consts_ap = nc.dram_tensor("consts", [P, 64], F32, kind="ExternalInput").ap()
    ones_ap = nc.dram_tensor("ones", [P, 1], F32R, kind="ExternalInput").ap()
    onesrow_ap = nc.dram_tensor("ones_row", [1, P], F32R, kind="ExternalInput").ap()
    fold_ap = nc.dram_tensor("fold", [1, 2 * D], BF16, kind="ExternalInput").ap()
    yT_ap = nc.dram_tensor("yT", [D, SQ], F32, kind="ExternalOutput").ap()
    aps = (xT_ap, wq_ap, wk_ap, wv_ap, wo_ap, w1_ap, w2_ap, consts_ap,
           ones_ap, onesrow_ap, fold_ap, yT_ap)
    from contextlib import ExitStack
    with tile.TileContext(nc) as tc, ExitStack() as ctx:
        _emit(ctx, tc, aps)
    nc.compile()
    return nc


_cached_nc = None


def _get_nc():
    global _cached_nc
    if _cached_nc is None:
        _cached_nc = build()
    return _cached_nc


def _to_bf16(a):
    return np.ascontiguousarray(np.asarray(a, np.float32)).astype(
        ml_dtypes.bfloat16)


def _to_fp8(a, scale):
    return np.clip(np.asarray(a, np.float32) * scale, -240.0, 240.0).astype(
        ml_dtypes.float8_e4m3)


def _prep_in_maps(x, Wq, Wk, Wv, Wo, bo, ln1_g, ln1_b, W1, b1, W2, b2,
                  ln2_g, ln2_b):
    f = np.float32
    consts = np.zeros((P, 64), f)
    consts[:, _C_BO:_C_BO + 8] = np.asarray(bo, f).reshape(8, P).T
    consts[:, _C_B1:_C_B1 + 16] = np.asarray(b1, f).reshape(16, P).T
    consts[:, _C_B2:_C_B2 + 8] = np.asarray(b2, f).reshape(8, P).T
    consts[:, _C_G1:_C_G1 + 8] = np.asarray(ln1_g, f).reshape(8, P).T
    consts[:, _C_BE1:_C_BE1 + 8] = np.asarray(ln1_b, f).reshape(8, P).T
    consts[:, _C_G2:_C_G2 + 8] = np.asarray(ln2_g, f).reshape(8, P).T
    consts[:, _C_BE2:_C_BE2 + 8] = np.asarray(ln2_b, f).reshape(8, P).T
    ones = np.ones((P, 1), f)
    ones_row = np.ones((1, P), f)
    W1f = np.asarray(W1, np.float64)
    W2f = np.asarray(W2, np.float64)
    g1v = np.asarray(ln1_g, np.float64)
    b1v = np.asarray(ln1_b, np.float64)
    c1 = np.asarray(b1, np.float64) + (b1v[:, None] * W1f).sum(axis=0)
    W1g = (g1v[:, None] * W1f).astype(f)
    w2g1 = (g1v[:, None] * W1f).sum(axis=0) @ W2f
    c2 = np.asarray(b2, np.float64) + c1 @ W2f
    fold = np.concatenate([w2g1, c2]).astype(f)[None, :]

    def pack_st(W, dtype_fn):
        # [D_in, N] -> [P, N/P, D_in/P, P] stationary tiles
        din, n = W.shape
        return np.ascontiguousarray(
            dtype_fn(np.asarray(W, f).reshape(din // P, P, n // P, P)
                     .transpose(1, 2, 0, 3)))

    shared = {
        "Wq": pack_st(np.asarray(Wq, f), _to_bf16),
        "Wk": pack_st(np.asarray(Wk, f), lambda a: _to_fp8(a, ALPHA)),
        "Wo": pack_st(np.asarray(Wo, f), lambda a: _to_fp8(a, ALPHA)),
        "W1": pack_st(W1g, _to_bf16),
        "W2": pack_st(np.asarray(W2, f), _to_bf16),
        "Wv": np.ascontiguousarray(
            _to_fp8(np.asarray(Wv, f).reshape(DT, P, 2, SQ)
                    .transpose(1, 2, 0, 3), ALPHA)),
        "consts": consts, "ones": ones, "ones_row": ones_row,
        "fold": _to_bf16(fold),
    }
    xt = np.asarray(x, f).transpose(0, 2, 1)  # [B, D, S]
    in_maps = []
    for core in range(NCORES):
        b, off = core // 2, (core % 2) * SQ
        if off == 0:
            xrot = xt[b]
        else:
            xrot = np.concatenate([xt[b][:, off:], xt[b][:, :off]], axis=1)
        xpk = np.ascontiguousarray(
            _to_bf16(xrot.reshape(DT, P, S).transpose(1, 0, 2)))
        in_maps.append(dict(shared, xT=xpk))
    return in_maps


def run(inputs, trace=False, tmpdir=None):
    """Run the kernel on 8 cores. Returns (y, BassKernelResults)."""
    nc = _get_nc()
    in_maps = _prep_in_maps(
        inputs["x"], inputs["Wq"], inputs["Wk"], inputs["Wv"], inputs["Wo"],
        inputs["bo"], inputs["ln1_g"], inputs["ln1_b"], inputs["W1"],
        inputs["b1"], inputs["W2"], inputs["b2"], inputs["ln2_g"],
        inputs["ln2_b"])
    try:
        res = bass_utils.run_bass_kernel_spmd(nc, in_maps, list(range(NCORES)),
                                              trace=trace, tmpdir=tmpdir)
    except Exception:
        import time as _time
        _time.sleep(2.0)
        res = bass_utils.run_bass_kernel_spmd(nc, in_maps, list(range(NCORES)),
                                              trace=trace, tmpdir=tmpdir)
    y = np.empty((B, S, D), np.float32)
    for core in range(NCORES):
        b, off = core // 2, (core % 2) * SQ
        y[b, off:off + SQ, :] = res.results[core]["yT"].T
    return y, res


def kernel(x, mask, Wq, Wk, Wv, Wo, bo, ln1_g, ln1_b, W1, b1, W2, b2,
           ln2_g, ln2_b):
    # mask is all-ones per the problem spec -> identity in the reference.
    y, _ = run(dict(x=x, Wq=Wq, Wk=Wk, Wv=Wv, Wo=Wo, bo=bo, ln1_g=ln1_g,
                    ln1_b=ln1_b, W1=W1, b1=b1, W2=W2, b2=b2, ln2_g=ln2_g,
                    ln2_b=ln2_b))
    return y


# revision 7
# speedup vs baseline: 1.1950x; 1.0124x over previous
"""Trainium2 Bass kernel for nn_EncoderLayer (B=4, S=1024, D=1024, H=16, FF=2048).

Sharding: 8 cores = 4 batches x 2 sequence-halves, each core redundantly
computes K/V for its whole batch and runs the layer for its own 512 query
rows. Odd cores see the sequence rotated by 512 (softmax over keys is
permutation-invariant).

Mixed-precision PE pipeline:
  - K/V projections, QK^T scores, exp, attn*V and the output projection run
    in fp8e4 (TRN E4M3) with MatmulPerfMode.DoubleRow (two 128-row
    contraction tiles per instruction = 2x PE throughput). fp8 weights are
    pre-scaled by 32 on the host; the 1/32 is folded into PSUM->SBUF copies.
  - Q projection and both FFN matmuls stay bf16: their error lands on the
    residual stream where fp8 blows the 2e-2 budget (measured 5e-2 all-fp8
    vs 3.2e-3 with these three in bf16).

Schedule: the softmax exp stream (~57us on the scalar engine) is the
second-longest pole after the PE, so scores for head-pair j are emitted
immediately after the K projection of tile j, with attn*V lagging one pair
(PV(j-1) fills the PE while the vector engine re-quantizes k8[j]). V(c=0)
is interleaved into the j=0 block, V(c=1) into the j=4 block (attn*V for
pairs 0-3 only reads heads 0-7 = the c=0 half of V).

Layernorm plumbing (feature-major activations, stats via ones-column
matmuls, gamma/beta folded into W1 + rank-1 W2 corrections) as in v1; the
LN column-sum matmuls are interleaved (lag-one) into the Wo/FFN2 loops and
the LN2 normalize reads the A/B broadcast PSUM directly and writes bf16.
"""

import sys
import types

import numpy as np
import ml_dtypes


def _shim_axon_hooks():
    try:
        import antenv.axon_hooks  # noqa: F401
    except Exception:
        mod = types.ModuleType("antenv.axon_hooks")
        mod.get_axon_ntff_profile_hook = lambda: None
        mod.set_axon_ntff_profile_hook = lambda h: None
        sys.modules["antenv.axon_hooks"] = mod


_shim_axon_hooks()

from concourse import bacc, mybir, tile  # noqa: E402
from concourse import bass_utils  # noqa: E402

F32 = mybir.dt.float32
F32R = mybir.dt.float32r
BF16 = mybir.dt.bfloat16
FP8 = mybir.dt.float8e4
AF = mybir.ActivationFunctionType
DR = mybir.MatmulPerfMode.DoubleRow

B, S, D, H, DH, FF = 4, 1024, 1024, 16, 64, 2048
SQ = 512
P = 128
DT = D // P
FT = FF // P
ST = S // P
NCORES = 8
EPS = 1e-6
SCALE = 1.0 / 32.0
ALPHA = 32.0

_C_BO = 0
_C_B1 = 8
_C_B2 = 24
_C_G1 = 32
_C_BE1 = 40
_C_G2 = 48
_C_BE2 = 56


def _emit(ctx, tc, aps):
    nc = tc.nc
    (xT_ap, wq_ap, wk_ap, wv_ap, wo_ap, w1_ap, w2_ap, consts_ap, ones_ap,
     onesrow_ap, fold_ap, yT_ap) = aps

    acts = ctx.enter_context(tc.tile_pool(name="acts", bufs=1))
    wts = ctx.enter_context(tc.tile_pool(name="wts", bufs=2))
    e2p = ctx.enter_context(tc.tile_pool(name="e2p", bufs=5))
    sc2 = ctx.enter_context(tc.tile_pool(name="sc2", bufs=2))
    sc1 = ctx.enter_context(tc.tile_pool(name="sc1", bufs=1))
    pp = ctx.enter_context(tc.tile_pool(name="pp", bufs=2, space="PSUM"))
    pvp = ctx.enter_context(tc.tile_pool(name="pvp", bufs=4, space="PSUM"))

    def wload(tag, src_ap, shape, dtype, nsplit=2):
        w = wts.tile(shape, dtype, tag=tag, name=tag)
        step = shape[1] // nsplit
        for q in range(nsplit):
            nc.sync.dma_start(w[:, q * step:(q + 1) * step, :],
                              src_ap[:, q * step:(q + 1) * step, :])
        return w

    # ---- input DMA, ordered so Q's first pair can start early ----
    wqa = wload("wqa", wq_ap[:, 0], [P, DT, P], BF16)
    wqb = wload("wqb", wq_ap[:, 1], [P, DT, P], BF16)
    xtb_a = acts.tile([P, 4, SQ], BF16, tag="xtb_a", name="xtb_a")
    nc.sync.dma_start(xtb_a[:, 0:2, :], xT_ap[:, 0:2, 0:SQ])
    nc.sync.dma_start(xtb_a[:, 2:4, :], xT_ap[:, 2:4, 0:SQ])
    xtb_b = acts.tile([P, 4, SQ], BF16, tag="xtb_b", name="xtb_b")
    nc.sync.dma_start(xtb_b[:, 0:2, :], xT_ap[:, 4:6, 0:SQ])
    nc.sync.dma_start(xtb_b[:, 2:4, :], xT_ap[:, 6:8, 0:SQ])
    xtb_hi = acts.tile([P, DT, SQ], BF16, tag="xtb_hi", name="xtb_hi")
    for q in range(4):
        nc.sync.dma_start(xtb_hi[:, 2 * q:2 * q + 2, :],
                          xT_ap[:, 2 * q:2 * q + 2, SQ:S])
    consts = acts.tile([P, 64], F32, tag="consts", name="consts")
    nc.sync.dma_start(consts[:], consts_ap[:])
    ones_r = acts.tile([P, 1], F32R, tag="ones", name="ones")
    nc.sync.dma_start(ones_r[:], ones_ap[:])
    ones_row = acts.tile([1, P], F32R, tag="ones_row", name="ones_row")
    nc.sync.dma_start(ones_row[:], onesrow_ap[:])
    fold = acts.tile([1, 2 * D], BF16, tag="fold", name="fold")
    nc.sync.dma_start(fold[:], fold_ap[:])

    def xlo(k):
        return xtb_a[:, k, :] if k < 4 else xtb_b[:, k - 4, :]

    # ---- Q projection (bf16) + q_res/q8 copies + xt8 conversion ----
    q_res = []
    q8 = acts.tile([P, DT, SQ], FP8, tag="q8", name="q8")
    xt8 = acts.tile([P, DT, S], FP8, tag="xt8", name="xt8")
    for j0 in range(0, DT, 2):
        wa = wqa if j0 == 0 else wload("wqa", wq_ap[:, j0], [P, DT, P], BF16)
        wb = wqb if j0 == 0 else wload("wqb", wq_ap[:, j0 + 1], [P, DT, P], BF16)
        ps = pp.tile([P, 2, SQ], F32, tag="ps2", name="ps2")
        for k in range(DT):
            nc.tensor.matmul(ps[:, 0, :], wa[:, k, :], xlo(k),
                             start=(k == 0), stop=(k == DT - 1))
            nc.tensor.matmul(ps[:, 1, :], wb[:, k, :], xlo(k),
                             start=(k == 0), stop=(k == DT - 1))
        for h in range(2):
            j = j0 + h
            qr = acts.tile([P, SQ], F32R, tag=f"qres{j}", name=f"qres{j}")
            nc.scalar.copy(qr[:], ps[:, h, :])
            q_res.append(qr)
            nc.vector.tensor_copy(q8[:, j, :], ps[:, h, :])
        # xt8 conversion interleaved: vector does the low half, scalar high
        for k in (j0, j0 + 1):
            nc.vector.tensor_copy(xt8[:, k, 0:SQ], xlo(k))
            nc.scalar.copy(xt8[:, k, SQ:S], xtb_hi[:, k, :])

    # ---- attention machinery ----
    vr8 = acts.tile([P, ST, H, DH + 1], FP8, tag="vr8", name="vr8")
    nc.vector.memset(vr8[:, :, :, DH:DH + 1], 1.0)
    attn8 = acts.tile([P, DT, SQ], FP8, tag="attn8", name="attn8")
    k8 = [None] * DT
    e2s = {}
    pvs = {}

    def kproj(j, w):
        ps = pp.tile([P, 2, SQ], F32, tag="ps2", name="ps2")
        for k in range(0, DT, 2):
            nc.tensor.matmul(ps[:, 0, :], w[:, k:k + 2, :],
                             xt8[:, k:k + 2, 0:SQ],
                             start=(k == 0), stop=(k == DT - 2), perf_mode=DR)
            nc.tensor.matmul(ps[:, 1, :], w[:, k:k + 2, :],
                             xt8[:, k:k + 2, SQ:S],
                             start=(k == 0), stop=(k == DT - 2), perf_mode=DR)
        kj = acts.tile([P, S], FP8, tag=f"k8{j}", name=f"k8{j}")
        nc.vector.tensor_scalar_mul(
            kj[:].rearrange("p (c q) -> p c q", c=2), ps[:], 1.0 / ALPHA)
        k8[j] = kj

    def v_group(c, st, wv):
        ps = pp.tile([P, 2, SQ], F32, tag="ps2", name="ps2")
        for k in range(0, DT, 2):
            nc.tensor.matmul(ps[:, 0, :],
                             xt8[:, k:k + 2, st * P:(st + 1) * P],
                             wv[:, k:k + 2, :],
                             start=(k == 0), stop=(k == DT - 2), perf_mode=DR)
        nc.vector.tensor_scalar_mul(
            vr8[:, st, c * 8:(c + 1) * 8, 0:DH],
            ps[:, 0, :].rearrange("p (h d) -> p h d", d=DH), 1.0 / ALPHA)

    def scx(j, st2):
        """scores + exp for head pair (2j, 2j+1), key tiles 2*st2, 2*st2+1."""
        e2 = e2p.tile([P, 2, 2, SQ], FP8, tag="e2", name="e2")
        for sti in range(2):
            st = 2 * st2 + sti
            sl = slice(st * P, (st + 1) * P)
            ps = pp.tile([P, 2, SQ], F32, tag="ps2", name="ps2")
            nc.tensor.matmul(ps[:, 0, :], k8[j][0:DH, sl], q8[0:DH, j, :],
                             start=True, stop=True)
            nc.tensor.matmul(ps[:, 1, :], k8[j][DH:P, sl], q8[DH:P, j, :],
                             start=True, stop=True)
            nc.scalar.activation(e2[:, sti, :, :], ps[:], AF.Exp, scale=SCALE)
        e2s[(j, st2)] = e2

    def pv_acc(j, st2):
        if st2 == 0:
            pvs[j] = (pvp.tile([DH + 1, SQ], F32, tag="pv", name="pv"),
                      pvp.tile([DH + 1, SQ], F32, tag="pv", name="pv"))
        pv0, pv1 = pvs[j]
        e2 = e2s.pop((j, st2))
        nc.tensor.matmul(pv0[:], vr8[:, 2 * st2:2 * st2 + 2, 2 * j, :],
                         e2[:, :, 0, :], start=(st2 == 0),
                         stop=(st2 == ST // 2 - 1), perf_mode=DR)
        nc.tensor.matmul(pv1[:], vr8[:, 2 * st2:2 * st2 + 2, 2 * j + 1, :],
                         e2[:, :, 1, :], start=(st2 == 0),
                         stop=(st2 == ST // 2 - 1), perf_mode=DR)

    def norm(j):
        pv0, pv1 = pvs.pop(j)
        for half, pv in ((0, pv0), (1, pv1)):
            rows = slice(half * DH, half * DH + DH)
            zh = sc2.tile([1, SQ], F32, tag="zh", name="zh")
            nc.vector.tensor_copy(zh[:], pv[DH:DH + 1, :])
            iz = sc2.tile([1, SQ], F32, tag="zh", name="iz")
            nc.vector.reciprocal_approx_fast(iz[:], zh[:])
            bz = sc2.tile([DH, SQ], F32, tag="sb", name="sb")
            nc.gpsimd.partition_broadcast(bz[:], iz[:])
            nc.vector.tensor_mul(attn8[rows, j, :], pv[0:DH, :], bz[:])

    # ---- K + scores/exp + V + attn*V, exp-first schedule ----
    wv0 = wload("wv", wv_ap[:, 0], [P, DT, SQ], FP8, nsplit=4)
    kproj(0, wload("wka", wk_ap[:, 0], [P, DT, P], FP8))
    for st2 in range(4):
        v_group(0, 2 * st2, wv0)
        v_group(0, 2 * st2 + 1, wv0)
        scx(0, st2)
    for j in range(1, DT):
        w = wload("wka" if j % 2 == 0 else "wkb", wk_ap[:, j], [P, DT, P], FP8)
        kproj(j, w)
        if j == 4:
            wv1 = wload("wv", wv_ap[:, 1], [P, DT, SQ], FP8, nsplit=4)
        for st2 in range(4):
            if j == 4:
                v_group(1, 2 * st2, wv1)
                v_group(1, 2 * st2 + 1, wv1)
            pv_acc(j - 1, st2)
            scx(j, st2)
        norm(j - 1)
    for st2 in range(4):
        pv_acc(DT - 1, st2)
    norm(DT - 1)

    # ---- output projection (fp8 DoubleRow) + relu + residual + LN1 stats ----
    h1, sq1 = [], []
    h18 = acts.tile([P, DT, SQ], BF16, tag="h18", name="h18")
    ps_sum1 = pvp.tile([1, SQ], F32, tag="pv", name="ps_sum1")
    ps_sq1 = pvp.tile([1, SQ], F32, tag="pv", name="ps_sq1")

    def colsum(ps_sum, ps_sq, src, sq, j):
        nc.tensor.matmul(ps_sum[:], ones_r[:], src[:],
                         start=(j == 0), stop=(j == DT - 1))
        nc.tensor.matmul(ps_sq[:], ones_r[:], sq[:],
                         start=(j == 0), stop=(j == DT - 1))

    for j0 in range(0, DT, 2):
        wa = wload("woa", wo_ap[:, j0], [P, DT, P], FP8)
        wb = wload("wob", wo_ap[:, j0 + 1], [P, DT, P], FP8)
        ps = pp.tile([P, 2, SQ], F32, tag="ps2", name="ps2")
        for k in range(0, DT, 2):
            nc.tensor.matmul(ps[:, 0, :], wa[:, k:k + 2, :],
                             attn8[:, k:k + 2, :],
                             start=(k == 0), stop=(k == DT - 2), perf_mode=DR)
            nc.tensor.matmul(ps[:, 1, :], wb[:, k:k + 2, :],
                             attn8[:, k:k + 2, :],
                             start=(k == 0), stop=(k == DT - 2), perf_mode=DR)
        # lag-one interleave of the LN1 column sums
        for j in (j0 - 2, j0 - 1):
            if j >= 0:
                colsum(ps_sum1, ps_sq1, h1[j], sq1[j], j)
        for h in range(2):
            j = j0 + h
            rel = sc2.tile([P, SQ], F32R, tag="u", name="rel")
            nc.scalar.activation(rel[:], ps[:, h, :], AF.Relu,
                                 bias=consts[:, _C_BO + j:_C_BO + j + 1],
                                 scale=1.0 / ALPHA)
            t = acts.tile([P, SQ], F32R, tag=f"h1{j}", name=f"h1_{j}")
            nc.vector.tensor_add(t[:], rel[:], q_res[j][:])
            h1.append(t)
            nc.scalar.copy(h18[:, j, :], t[:])
            sq = acts.tile([P, SQ], F32R, tag=f"sq1{j}", name=f"sq1_{j}")
            nc.vector.tensor_mul(sq[:], t[:], t[:])
            sq1.append(sq)
    for j in (DT - 2, DT - 1):
        colsum(ps_sum1, ps_sq1, h1[j], sq1[j], j)

    # LN1 chain (gamma/beta folded into W1 / rank-1 W2 fold)
    s_sb = sc1.tile([1, SQ], F32, tag="s0", name="s_sb")
    nc.vector.tensor_copy(s_sb[:], ps_sum1[:])
    m2 = sc1.tile([1, SQ], F32, tag="s1", name="m2")
    nc.vector.tensor_mul(m2[:], s_sb[:], s_sb[:])
    a_t = sc1.tile([1, SQ], F32, tag="s2", name="a_t")
    nc.vector.scalar_tensor_tensor(a_t[:], m2[:], 1.0 / D, ps_sq1[:],
                                   op0=mybir.AluOpType.mult,
                                   op1=mybir.AluOpType.subtract)
    eps_t = sc1.tile([1, 1], F32, tag="eps", name="eps")
    nc.vector.memset(eps_t[:], EPS)
    sd1 = sc1.tile([1, SQ], F32, tag="s1", name="sd1")
    nc.scalar.activation(sd1[:], a_t[:], AF.Sqrt, bias=eps_t[:], scale=-1.0 / D)
    rstd1 = sc1.tile([1, SQ], F32, tag="s2", name="rstd1")
    nc.vector.reciprocal_approx_fast(rstd1[:], sd1[:])
    bneg1 = sc1.tile([1, SQ], F32, tag="s3", name="bneg1")
    nc.vector.scalar_tensor_tensor(bneg1[:], s_sb[:], -1.0 / D, rstd1[:],
                                   op0=mybir.AluOpType.mult,
                                   op1=mybir.AluOpType.mult)
    negmu_r = sc1.tile([1, SQ], BF16, tag="s4", name="negmu_r")
    nc.vector.tensor_scalar_mul(negmu_r[:], s_sb[:], -1.0 / D)
    sd_r = sc1.tile([1, SQ], BF16, tag="s5", name="sd_r")
    nc.vector.tensor_copy(sd_r[:], sd1[:])
    abc_sb = sc2.tile([P, SQ], F32, tag="sb", name="abc_sb")
    nc.gpsimd.partition_broadcast(abc_sb[:], rstd1[:])
    bbc_sb = sc2.tile([P, SQ], F32, tag="zh", name="bbc_sb")
    nc.gpsimd.partition_broadcast(bbc_sb[:], bneg1[:])

    # ---- FFN1 (bf16) ----
    hid_a = acts.tile([P, DT, SQ], BF16, tag="xtb_hi", name="hid_a")
    hid_b = acts.tile([P, DT, SQ], BF16, tag="hid_b", name="hid_b")
    for f0 in range(0, FT, 2):
        wa = wload("w1a", w1_ap[:, f0], [P, DT, P], BF16)
        wb = wload("w1b", w1_ap[:, f0 + 1], [P, DT, P], BF16)
        ps = pp.tile([P, 2, SQ], F32, tag="ps2", name="ps2")
        for k in range(DT):
            nc.tensor.matmul(ps[:, 0, :], wa[:, k, :], h18[:, k, :],
                             start=(k == 0), stop=(k == DT - 1))
            nc.tensor.matmul(ps[:, 1, :], wb[:, k, :], h18[:, k, :],
                             start=(k == 0), stop=(k == DT - 1))
        hid = hid_a if f0 < DT else hid_b
        nc.scalar.copy(hid[:, f0 % DT:f0 % DT + 2, :], ps[:])

    # real ln1 for the residual (overlaps FFN1)
    ln1 = []
    for j in range(DT):
        u = sc2.tile([P, SQ], F32, tag="u", name="u")
        nc.vector.tensor_mul(u[:], h1[j][:], abc_sb[:])
        nc.vector.tensor_add(u[:], u[:], bbc_sb[:])
        d = acts.tile([P, SQ], F32R, tag=f"ln1{j}", name=f"ln1_{j}")
        nc.scalar.activation(d[:], u[:], AF.Identity,
                             bias=consts[:, _C_BE1 + j:_C_BE1 + j + 1],
                             scale=consts[:, _C_G1 + j:_C_G1 + j + 1])
        ln1.append(d)

    # ---- FFN2 (bf16) + rank-1 LN1 fold + relu + residual + LN2 stats ----
    f2, sq2 = [], []
    ps_sum2 = pvp.tile([1, SQ], F32, tag="pv", name="ps_sum2")
    ps_sq2 = pvp.tile([1, SQ], F32, tag="pv", name="ps_sq2")
    for j in range(DT):
        w = wload("w2", w2_ap[:, j], [P, FT, P], BF16, nsplit=4)
        ps = pp.tile([P, 2, SQ], F32, tag="ps2", name="ps2")
        for f in range(FT):
            hid = hid_a if f < DT else hid_b
            nc.tensor.matmul(ps[:, 0, :], w[:, f, :], hid[:, f % DT, :],
                             start=(f == 0), stop=False)
        nc.tensor.matmul(ps[:, 0, :], fold[0:1, j * P:(j + 1) * P],
                         negmu_r[:], start=False, stop=False)
        nc.tensor.matmul(ps[:, 0, :], fold[0:1, D + j * P:D + (j + 1) * P],
                         sd_r[:], start=False, stop=True)
        if j > 0:
            colsum(ps_sum2, ps_sq2, f2[j - 1], sq2[j - 1], j - 1)
        rel = sc2.tile([P, SQ], F32R, tag="u", name="rel2")
        nc.vector.scalar_tensor_tensor(rel[:], ps[:, 0, :], 0.0, abc_sb[:],
                                       op0=mybir.AluOpType.max,
                                       op1=mybir.AluOpType.mult)
        t = acts.tile([P, SQ], F32R, tag=f"h1{j}", name=f"f2_{j}")
        nc.vector.tensor_add(t[:], rel[:], ln1[j][:])
        f2.append(t)
        sq = acts.tile([P, SQ], F32R, tag=f"sq1{j}", name=f"sq2_{j}")
        nc.scalar.activation(sq[:], t[:], AF.Square)
        sq2.append(sq)
    colsum(ps_sum2, ps_sq2, f2[DT - 1], sq2[DT - 1], DT - 1)

    # ---- LN2 chain + normalize (writes bf16, DMA per tile) ----
    s_sb2 = sc1.tile([1, SQ], F32, tag="s0", name="s_sb2")
    nc.vector.tensor_copy(s_sb2[:], ps_sum2[:])
    m22 = sc1.tile([1, SQ], F32, tag="s1", name="m22")
    nc.vector.tensor_mul(m22[:], s_sb2[:], s_sb2[:])
    a_t2 = sc1.tile([1, SQ], F32, tag="s2", name="a_t2")
    nc.vector.scalar_tensor_tensor(a_t2[:], m22[:], 1.0 / D, ps_sq2[:],
                                   op0=mybir.AluOpType.mult,
                                   op1=mybir.AluOpType.subtract)
    sd2 = sc1.tile([1, SQ], F32, tag="s1", name="sd2")
    nc.scalar.activation(sd2[:], a_t2[:], AF.Sqrt, bias=eps_t[:],
                         scale=-1.0 / D)
    rstd2 = sc1.tile([1, SQ], F32, tag="s2", name="rstd2")
    nc.vector.reciprocal_approx_fast(rstd2[:], sd2[:])
    bneg2 = sc1.tile([1, SQ], F32, tag="s3", name="bneg2")
    nc.vector.scalar_tensor_tensor(bneg2[:], s_sb2[:], -1.0 / D, rstd2[:],
                                   op0=mybir.AluOpType.mult,
                                   op1=mybir.AluOpType.mult)
    a_r = sc1.tile([1, SQ], F32R, tag="s0", name="a_r2")
    nc.vector.tensor_copy(a_r[:], rstd2[:])
    b_r = sc1.tile([1, SQ], F32R, tag="s4", name="b_r2")
    nc.vector.tensor_copy(b_r[:], bneg2[:])
    ab = pp.tile([P, 2, SQ], F32, tag="ps2", name="ab")
    nc.tensor.matmul(ab[:, 0, :], ones_row[:], a_r[:], start=True, stop=True)
    nc.tensor.matmul(ab[:, 1, :], ones_row[:], b_r[:], start=True, stop=True)
    # SBUF copies of A/B for the gpsimd-normalized tiles (gpsimd can't read
    # PSUM); the vector-normalized tiles read the PSUM broadcast directly.
    a_sb2 = sc2.tile([P, SQ], F32, tag="sb", name="a_sb2")
    nc.scalar.copy(a_sb2[:], ab[:, 0, :])
    b_sb2 = sc2.tile([P, SQ], F32, tag="zh", name="b_sb2")
    nc.scalar.copy(b_sb2[:], ab[:, 1, :])
    for j in range(DT):
        gp = j % 4 == 3
        eng = nc.gpsimd if gp else nc.vector
        u = sc2.tile([P, SQ], F32, tag="u", name="u")
        eng.tensor_mul(u[:], f2[j][:], a_sb2[:] if gp else ab[:, 0, :])
        eng.tensor_add(u[:], u[:], b_sb2[:] if gp else ab[:, 1, :])
        d = acts.tile([P, SQ], BF16, tag=f"qres{j}", name=f"y_{j}")
        nc.scalar.activation(d[:], u[:], AF.Identity,
                             bias=consts[:, _C_BE2 + j:_C_BE2 + j + 1],
                             scale=consts[:, _C_G2 + j:_C_G2 + j + 1])
        nc.sync.dma_start(yT_ap[j * P:(j + 1) * P, :], d[:])


def build():
    nc = bacc.Bacc("TRN2", target_bir_lowering=False, debug=False,
                   num_devices=NCORES)
    xT_ap = nc.dram_tensor("xT", [P, DT, S], BF16, kind="ExternalInput").ap()
    wq_ap = nc.dram_tensor("Wq", [P, DT, DT, P], BF16, kind="ExternalInput").ap()
    wk_ap = nc.dram_tensor("Wk", [P, DT, DT, P], FP8, kind="ExternalInput").ap()
    wv_ap = nc.dram_tensor("Wv", [P, 2, DT, SQ], FP8, kind="ExternalInput").ap()
    wo_ap = nc.dram_tensor("Wo", [P, DT, DT, P], FP8, kind="ExternalInput").ap()
    w1_ap = nc.dram_tensor("W1", [P, FT, DT, P], BF16, kind="ExternalInput").ap()
    w2_ap = nc.dram_tensor("W2", [P, DT, FT, P], BF16, kind="ExternalInput").ap()
    consts_ap = nc.dram_tensor("consts", [P, 64], F32, kind="ExternalInput").ap()
    ones_ap = nc.dram_tensor("ones", [P, 1], F32R, kind="ExternalInput").ap()
    onesrow_ap = nc.dram_tensor("ones_row", [1, P], F32R, kind="ExternalInput").ap()
    fold_ap = nc.dram_tensor("fold", [1, 2 * D], BF16, kind="ExternalInput").ap()
    yT_ap = nc.dram_tensor("yT", [D, SQ], BF16, kind="ExternalOutput").ap()
    aps = (xT_ap, wq_ap, wk_ap, wv_ap, wo_ap, w1_ap, w2_ap, consts_ap,
           ones_ap, onesrow_ap, fold_ap, yT_ap)
    from contextlib import ExitStack
    with tile.TileContext(nc) as tc, ExitStack() as ctx:
        _emit(ctx, tc, aps)
    nc.compile()
    return nc


_cached_nc = None


def _get_nc():
    global _cached_nc
    if _cached_nc is None:
        _cached_nc = build()
    return _cached_nc


def _to_bf16(a):
    return np.ascontiguousarray(np.asarray(a, np.float32)).astype(
        ml_dtypes.bfloat16)


def _to_fp8(a, scale):
    return np.clip(np.asarray(a, np.float32) * scale, -240.0, 240.0).astype(
        ml_dtypes.float8_e4m3)


def _prep_in_maps(x, Wq, Wk, Wv, Wo, bo, ln1_g, ln1_b, W1, b1, W2, b2,
                  ln2_g, ln2_b):
    f = np.float32
    consts = np.zeros((P, 64), f)
    consts[:, _C_BO:_C_BO + 8] = np.asarray(bo, f).reshape(8, P).T
    consts[:, _C_B1:_C_B1 + 16] = np.asarray(b1, f).reshape(16, P).T
    consts[:, _C_B2:_C_B2 + 8] = np.asarray(b2, f).reshape(8, P).T
    consts[:, _C_G1:_C_G1 + 8] = np.asarray(ln1_g, f).reshape(8, P).T
    consts[:, _C_BE1:_C_BE1 + 8] = np.asarray(ln1_b, f).reshape(8, P).T
    consts[:, _C_G2:_C_G2 + 8] = np.asarray(ln2_g, f).reshape(8, P).T
    consts[:, _C_BE2:_C_BE2 + 8] = np.asarray(ln2_b, f).reshape(8, P).T
    ones = np.ones((P, 1), f)
    ones_row = np.ones((1, P), f)
    W1f = np.asarray(W1, np.float64)
    W2f = np.asarray(W2, np.float64)
    g1v = np.asarray(ln1_g, np.float64)
    b1v = np.asarray(ln1_b, np.float64)
    c1 = np.asarray(b1, np.float64) + (b1v[:, None] * W1f).sum(axis=0)
    W1g = (g1v[:, None] * W1f).astype(f)
    w2g1 = (g1v[:, None] * W1f).sum(axis=0) @ W2f
    c2 = np.asarray(b2, np.float64) + c1 @ W2f
    fold = np.concatenate([w2g1, c2]).astype(f)[None, :]

    def pack_st(W, dtype_fn):
        # [D_in, N] -> [P, N/P, D_in/P, P] stationary tiles
        din, n = W.shape
        return np.ascontiguousarray(
            dtype_fn(np.asarray(W, f).reshape(din // P, P, n // P, P)
                     .transpose(1, 2, 0, 3)))

    shared = {
        "Wq": pack_st(np.asarray(Wq, f), _to_bf16),
        "Wk": pack_st(np.asarray(Wk, f), lambda a: _to_fp8(a, ALPHA)),
        "Wo": pack_st(np.asarray(Wo, f), lambda a: _to_fp8(a, ALPHA)),
        "W1": pack_st(W1g, _to_bf16),
        "W2": pack_st(np.asarray(W2, f), _to_bf16),
        "Wv": np.ascontiguousarray(
            _to_fp8(np.asarray(Wv, f).reshape(DT, P, 2, SQ)
                    .transpose(1, 2, 0, 3), ALPHA)),
        "consts": consts, "ones": ones, "ones_row": ones_row,
        "fold": _to_bf16(fold),
    }
    xt = np.asarray(x, f).transpose(0, 2, 1)  # [B, D, S]
    in_maps = []
    for core in range(NCORES):
        b, off = core // 2, (core % 2) * SQ
        if off == 0:
            xrot = xt[b]
        else:
            xrot = np.concatenate([xt[b][:, off:], xt[b][:, :off]], axis=1)
        xpk = np.ascontiguousarray(
            _to_bf16(xrot.reshape(DT, P, S).transpose(1, 0, 2)))
        in_maps.append(dict(shared, xT=xpk))
    return in_maps


def run(inputs, trace=False, tmpdir=None):
    """Run the kernel on 8 cores. Returns (y, BassKernelResults)."""
    nc = _get_nc()
    in_maps = _prep_in_maps(
        inputs["x"], inputs["Wq"], inputs["Wk"], inputs["Wv"], inputs["Wo"],
        inputs["bo"], inputs["ln1_g"], inputs["ln1_b"], inputs["W1"],
        inputs["b1"], inputs["W2"], inputs["b2"], inputs["ln2_g"],
        inputs["ln2_b"])
    try:
        res = bass_utils.run_bass_kernel_spmd(nc, in_maps, list(range(NCORES)),
                                              trace=trace, tmpdir=tmpdir)
    except Exception:
        import time as _time
        _time.sleep(2.0)
        res = bass_utils.run_bass_kernel_spmd(nc, in_maps, list(range(NCORES)),
                                              trace=trace, tmpdir=tmpdir)
    y = np.empty((B, S, D), np.float32)
    for core in range(NCORES):
        b, off = core // 2, (core % 2) * SQ
        y[b, off:off + SQ, :] = res.results[core]["yT"].astype(np.float32).T
    return y, res


def kernel(x, mask, Wq, Wk, Wv, Wo, bo, ln1_g, ln1_b, W1, b1, W2, b2,
           ln2_g, ln2_b):
    # mask is all-ones per the problem spec -> identity in the reference.
    y, _ = run(dict(x=x, Wq=Wq, Wk=Wk, Wv=Wv, Wo=Wo, bo=bo, ln1_g=ln1_g,
                    ln1_b=ln1_b, W1=W1, b1=b1, W2=W2, b2=b2, ln2_g=ln2_g,
                    ln2_b=ln2_b))
    return y


# revision 12
# speedup vs baseline: 1.2012x; 1.0052x over previous
"""Trainium2 Bass kernel for nn_EncoderLayer (B=4, S=1024, D=1024, H=16, FF=2048).

Sharding: 8 cores = 4 batches x 2 sequence-halves, each core redundantly
computes K/V for its whole batch and runs the layer for its own 512 query
rows. Odd cores see the sequence rotated by 512 (softmax over keys is
permutation-invariant).

Mixed-precision PE pipeline:
  - K/V projections, QK^T scores, exp, attn*V and the output projection run
    in fp8e4 (TRN E4M3) with MatmulPerfMode.DoubleRow (two 128-row
    contraction tiles per instruction = 2x PE throughput). fp8 weights are
    pre-scaled by 32 on the host; the 1/32 is folded into PSUM->SBUF copies.
  - Q projection and both FFN matmuls stay bf16: their error lands on the
    residual stream where fp8 blows the 2e-2 budget (measured 5e-2 all-fp8
    vs 3.2e-3 with these three in bf16).

Schedule: the softmax exp stream (~57us on the scalar engine) is the
second-longest pole after the PE, so scores for head-pair j are emitted
immediately after the K projection of tile j, with attn*V lagging one pair
(PV(j-1) fills the PE while the vector engine re-quantizes k8[j]). V(c=0)
is interleaved into the j=0 block, V(c=1) into the j=4 block (attn*V for
pairs 0-3 only reads heads 0-7 = the c=0 half of V).

Layernorm plumbing (feature-major activations, stats via ones-column
matmuls, gamma/beta folded into W1 + rank-1 W2 corrections) as in v1; the
LN column-sum matmuls are interleaved (lag-one) into the Wo/FFN2 loops and
the LN2 normalize reads the A/B broadcast PSUM directly and writes bf16.
"""

import sys
import types

import numpy as np
import ml_dtypes


def _shim_axon_hooks():
    try:
        import antenv.axon_hooks  # noqa: F401
    except Exception:
        mod = types.ModuleType("antenv.axon_hooks")
        mod.get_axon_ntff_profile_hook = lambda: None
        mod.set_axon_ntff_profile_hook = lambda h: None
        sys.modules["antenv.axon_hooks"] = mod


_shim_axon_hooks()

from concourse import bacc, mybir, tile  # noqa: E402
from concourse import bass_utils  # noqa: E402

F32 = mybir.dt.float32
F32R = mybir.dt.float32r
BF16 = mybir.dt.bfloat16
FP8 = mybir.dt.float8e4
AF = mybir.ActivationFunctionType
DR = mybir.MatmulPerfMode.DoubleRow

B, S, D, H, DH, FF = 4, 1024, 1024, 16, 64, 2048
SQ = 512
P = 128
DT = D // P
FT = FF // P
ST = S // P
NCORES = 8
EPS = 1e-6
SCALE = 1.0 / 32.0
ALPHA = 32.0

_C_BO = 0
_C_B1 = 8
_C_B2 = 24
_C_G1 = 32
_C_BE1 = 40
_C_G2 = 48
_C_BE2 = 56


def _emit(ctx, tc, aps):
    nc = tc.nc
    (xT_ap, x8_ap, wq_ap, wk_ap, wv_ap, wo_ap, w1_ap, w2_ap, consts_ap,
     ones_ap, onesrow_ap, fold_ap, yT_ap) = aps

    acts = ctx.enter_context(tc.tile_pool(name="acts", bufs=1))
    wts = ctx.enter_context(tc.tile_pool(name="wts", bufs=2))
    e2p = ctx.enter_context(tc.tile_pool(name="e2p", bufs=8))
    sc2 = ctx.enter_context(tc.tile_pool(name="sc2", bufs=2))
    sc1 = ctx.enter_context(tc.tile_pool(name="sc1", bufs=1))
    pp = ctx.enter_context(tc.tile_pool(name="pp", bufs=1, space="PSUM"))
    pss = ctx.enter_context(tc.tile_pool(name="pss", bufs=2, space="PSUM"))
    pvp = ctx.enter_context(tc.tile_pool(name="pvp", bufs=2, space="PSUM"))

    def wload(tag, src_ap, shape, dtype, nsplit=2):
        w = wts.tile(shape, dtype, tag=tag, name=tag)
        step = shape[1] // nsplit
        for q in range(nsplit):
            nc.sync.dma_start(w[:, q * step:(q + 1) * step, :],
                              src_ap[:, q * step:(q + 1) * step, :])
        return w

    # ---- input DMA: xt8 + wk0 first (K0 gates the exp stream), then Q's ----
    xt8 = acts.tile([P, DT, S], FP8, tag="xt8", name="xt8")
    for q in range(4):
        nc.sync.dma_start(xt8[:, 2 * q:2 * q + 2, :],
                          x8_ap[:, 2 * q:2 * q + 2, :])
    wk0 = wload("wka", wk_ap[:, 0], [P, DT, P], FP8)
    wqa = wload("wqa", wq_ap[:, 0], [P, DT, P], BF16)
    wqb = wload("wqb", wq_ap[:, 1], [P, DT, P], BF16)
    xtb_a = acts.tile([P, 4, SQ], BF16, tag="xtb_a", name="xtb_a")
    nc.sync.dma_start(xtb_a[:, 0:2, :], xT_ap[:, 0:2, 0:SQ])
    nc.sync.dma_start(xtb_a[:, 2:4, :], xT_ap[:, 2:4, 0:SQ])
    xtb_b = acts.tile([P, 4, SQ], BF16, tag="xtb_b", name="xtb_b")
    nc.sync.dma_start(xtb_b[:, 0:2, :], xT_ap[:, 4:6, 0:SQ])
    nc.sync.dma_start(xtb_b[:, 2:4, :], xT_ap[:, 6:8, 0:SQ])
    consts = acts.tile([P, 64], F32, tag="consts", name="consts")
    nc.sync.dma_start(consts[:], consts_ap[:])
    ones_r = acts.tile([P, 1], F32R, tag="ones", name="ones")
    nc.sync.dma_start(ones_r[:], ones_ap[:])
    ones_row = acts.tile([1, P], F32R, tag="ones_row", name="ones_row")
    nc.sync.dma_start(ones_row[:], onesrow_ap[:])
    ones_b = acts.tile([P, 1], BF16, tag="ones_b", name="ones_b")
    nc.vector.memset(ones_b[:], 1.0)
    fold = acts.tile([1, 2 * D], BF16, tag="fold", name="fold")
    nc.sync.dma_start(fold[:], fold_ap[:])

    def xlo(k):
        return xtb_a[:, k, :] if k < 4 else xtb_b[:, k - 4, :]

    # ---- attention machinery ----
    q_res = []
    q8 = acts.tile([P, DT, SQ], FP8, tag="q8", name="q8")
    vr8 = acts.tile([P, ST, H, DH + 1], FP8, tag="vr8", name="vr8")
    nc.vector.memset(vr8[:, :, :, DH:DH + 1], 1.0)
    attn8 = acts.tile([P, DT, SQ], FP8, tag="attn8", name="attn8")
    k8 = [None] * DT
    e2s = {}
    pvs = {}

    def qpair(j0, wa, wb):
        ps = pp.tile([P, 2, SQ], F32, tag="ps2", name="ps2")
        for k in range(DT):
            nc.tensor.matmul(ps[:, 0, :], wa[:, k, :], xlo(k),
                             start=(k == 0), stop=(k == DT - 1))
            nc.tensor.matmul(ps[:, 1, :], wb[:, k, :], xlo(k),
                             start=(k == 0), stop=(k == DT - 1))
        for h in range(2):
            j = j0 + h
            qr = acts.tile([P, SQ], F32R, tag=f"qres{j}", name=f"qres{j}")
            nc.scalar.copy(qr[:], ps[:, h, :])
            q_res.append(qr)
            nc.vector.tensor_copy(q8[:, j, :], ps[:, h, :])

    def kproj(j, w):
        ps = pp.tile([P, 2, SQ], F32, tag="ps2", name="ps2")
        for k in range(0, DT, 2):
            nc.tensor.matmul(ps[:, 0, :], w[:, k:k + 2, :],
                             xt8[:, k:k + 2, 0:SQ],
                             start=(k == 0), stop=(k == DT - 2), perf_mode=DR)
            nc.tensor.matmul(ps[:, 1, :], w[:, k:k + 2, :],
                             xt8[:, k:k + 2, SQ:S],
                             start=(k == 0), stop=(k == DT - 2), perf_mode=DR)
        kj = acts.tile([P, S], FP8, tag=f"k8{j}", name=f"k8{j}")
        nc.vector.tensor_scalar_mul(
            kj[:].rearrange("p (c q) -> p c q", c=2), ps[:], 1.0 / ALPHA)
        k8[j] = kj

    def v_group(c, st, wv):
        ps = pp.tile([P, 2, SQ], F32, tag="ps2", name="ps2")
        for k in range(0, DT, 2):
            nc.tensor.matmul(ps[:, 0, :],
                             xt8[:, k:k + 2, st * P:(st + 1) * P],
                             wv[:, k:k + 2, :],
                             start=(k == 0), stop=(k == DT - 2), perf_mode=DR)
        nc.vector.tensor_scalar_mul(
            vr8[:, st, c * 8:(c + 1) * 8, 0:DH],
            ps[:, 0, :].rearrange("p (h d) -> p h d", d=DH), 1.0 / ALPHA)

    def scx(j, st2):
        """scores + exp for head pair (2j, 2j+1), key tiles 2*st2, 2*st2+1."""
        e2 = e2p.tile([P, 2, 2, SQ], FP8, tag="e2", name="e2")
        for sti in range(2):
            st = 2 * st2 + sti
            sl = slice(st * P, (st + 1) * P)
            ps = pss.tile([P, 2, SQ], F32, tag="pss", name="pss")
            nc.tensor.matmul(ps[:, 0, :], k8[j][0:DH, sl], q8[0:DH, j, :],
                             start=True, stop=True)
            nc.tensor.matmul(ps[:, 1, :], k8[j][DH:P, sl], q8[DH:P, j, :],
                             start=True, stop=True)
            nc.scalar.activation(e2[:, sti, :, :], ps[:], AF.Exp, scale=SCALE)
        e2s[(j, st2)] = e2

    def pv_acc(j, st2):
        if st2 == 0:
            pvs[j] = (pvp.tile([DH + 1, SQ], F32, tag="pv", name="pv"),
                      pvp.tile([DH + 1, SQ], F32, tag="pv", name="pv"))
        pv0, pv1 = pvs[j]
        e2 = e2s.pop((j, st2))
        nc.tensor.matmul(pv0[:], vr8[:, 2 * st2:2 * st2 + 2, 2 * j, :],
                         e2[:, :, 0, :], start=(st2 == 0),
                         stop=(st2 == ST // 2 - 1), perf_mode=DR)
        nc.tensor.matmul(pv1[:], vr8[:, 2 * st2:2 * st2 + 2, 2 * j + 1, :],
                         e2[:, :, 1, :], start=(st2 == 0),
                         stop=(st2 == ST // 2 - 1), perf_mode=DR)

    def pv_all(j):
        for st2 in range(4):
            pv_acc(j, st2)

    def norm(j):
        pv0, pv1 = pvs.pop(j)
        for half, pv in ((0, pv0), (1, pv1)):
            rows = slice(half * DH, half * DH + DH)
            zh = sc2.tile([1, SQ], F32, tag="zh", name="zh")
            nc.vector.tensor_copy(zh[:], pv[DH:DH + 1, :])
            iz = sc2.tile([1, SQ], F32, tag="zh", name="iz")
            nc.vector.reciprocal_approx_fast(iz[:], zh[:])
            bz = sc2.tile([DH, SQ], F32, tag="sb", name="sb")
            nc.gpsimd.partition_broadcast(bz[:], iz[:])
            nc.vector.tensor_mul(attn8[rows, j, :], pv[0:DH, :], bz[:])

    # ---- block schedule: K(j) first so exp starts ~9us in; Q pairs and V
    # groups fill the PE between the exp-gated scores/PV chains. V(c=0)
    # completes by block 2 (pv pairs 0-3 read heads 0-7), V(c=1) by block 5.
    # attn*V for pair p runs 1-2 blocks after its exps; e2 pool depth 8
    # covers the two in-flight pairs. ----
    wv0 = wload("wv", wv_ap[:, 0], [P, DT, SQ], FP8, nsplit=4)
    # blocks: per j, a list of PE work items before its scores
    kproj(0, wk0)
    qpair(0, wqa, wqb)
    v_group(0, 0, wv0)
    v_group(0, 1, wv0)
    for st2 in range(4):
        scx(0, st2)
    wv1 = None
    for j in range(1, DT):
        w = wload("wka" if j % 2 == 0 else "wkb", wk_ap[:, j], [P, DT, P], FP8)
        kproj(j, w)
        if j == 3:
            wv1 = wload("wv", wv_ap[:, 1], [P, DT, SQ], FP8, nsplit=4)
        # V groups for this block: c0 spread over blocks 1-2, c1 over 3-5
        vg = {1: [(0, 2), (0, 3), (0, 4), (0, 5)],
              2: [(0, 6), (0, 7)],
              3: [(1, 0), (1, 1)],
              4: [(1, 2), (1, 3)],
              5: [(1, 4), (1, 5), (1, 6), (1, 7)]}.get(j, [])
        for c, st in vg:
            v_group(c, st, wv0 if c == 0 else wv1)
        if j < 4:
            qpair(2 * j, wload("wqa", wq_ap[:, 2 * j], [P, DT, P], BF16),
                  wload("wqb", wq_ap[:, 2 * j + 1], [P, DT, P], BF16))
        if j >= 2:
            pv_all(j - 2)
            norm(j - 2)
        for st2 in range(4):
            scx(j, st2)
    for j in range(DT - 2, DT):
        pv_all(j)
        norm(j)

    # ---- output projection (fp8 DoubleRow) + relu + residual + LN1 stats ----
    h1, sq1 = [], []
    h18 = acts.tile([P, DT, SQ], BF16, tag="h18", name="h18")
    ps_sum1 = pvp.tile([1, SQ], F32, tag="pv", name="ps_sum1")
    ps_sq1 = pvp.tile([1, SQ], F32, tag="pv", name="ps_sq1")

    def colsum(ps_sum, ps_sq, src, sq, j):
        nc.tensor.matmul(ps_sum[:], ones_r[:], src[:],
                         start=(j == 0), stop=(j == DT - 1))
        nc.tensor.matmul(ps_sq[:], ones_b[:], sq[:],
                         start=(j == 0), stop=(j == DT - 1))

    for j0 in range(0, DT, 2):
        wa = wload("woa", wo_ap[:, j0], [P, DT, P], FP8)
        wb = wload("wob", wo_ap[:, j0 + 1], [P, DT, P], FP8)
        ps = pp.tile([P, 2, SQ], F32, tag="ps2", name="ps2")
        for k in range(0, DT, 2):
            nc.tensor.matmul(ps[:, 0, :], wa[:, k:k + 2, :],
                             attn8[:, k:k + 2, :],
                             start=(k == 0), stop=(k == DT - 2), perf_mode=DR)
            nc.tensor.matmul(ps[:, 1, :], wb[:, k:k + 2, :],
                             attn8[:, k:k + 2, :],
                             start=(k == 0), stop=(k == DT - 2), perf_mode=DR)
        # lag-one interleave of the LN1 column sums
        for j in (j0 - 2, j0 - 1):
            if j >= 0:
                colsum(ps_sum1, ps_sq1, h1[j], sq1[j], j)
        for h in range(2):
            j = j0 + h
            rel = sc2.tile([P, SQ], F32R, tag="u", name="rel")
            nc.scalar.activation(rel[:], ps[:, h, :], AF.Relu,
                                 bias=consts[:, _C_BO + j:_C_BO + j + 1],
                                 scale=1.0 / ALPHA)
            t = acts.tile([P, SQ], F32R, tag=f"h1{j}", name=f"h1_{j}")
            nc.vector.tensor_add(t[:], rel[:], q_res[j][:])
            h1.append(t)
            nc.scalar.copy(h18[:, j, :], t[:])
            sq = acts.tile([P, SQ], BF16, tag=f"sq1{j}", name=f"sq1_{j}")
            nc.vector.tensor_mul(sq[:], t[:], t[:])
            sq1.append(sq)
    for j in (DT - 2, DT - 1):
        colsum(ps_sum1, ps_sq1, h1[j], sq1[j], j)

    # LN1 chain (gamma/beta folded into W1 / rank-1 W2 fold)
    s_sb = sc1.tile([1, SQ], F32, tag="s0", name="s_sb")
    nc.vector.tensor_copy(s_sb[:], ps_sum1[:])
    m2 = sc1.tile([1, SQ], F32, tag="s1", name="m2")
    nc.vector.tensor_mul(m2[:], s_sb[:], s_sb[:])
    a_t = sc1.tile([1, SQ], F32, tag="s2", name="a_t")
    nc.vector.scalar_tensor_tensor(a_t[:], m2[:], 1.0 / D, ps_sq1[:],
                                   op0=mybir.AluOpType.mult,
                                   op1=mybir.AluOpType.subtract)
    eps_t = sc1.tile([1, 1], F32, tag="eps", name="eps")
    nc.vector.memset(eps_t[:], EPS)
    sd1 = sc1.tile([1, SQ], F32, tag="s1", name="sd1")
    nc.scalar.activation(sd1[:], a_t[:], AF.Sqrt, bias=eps_t[:], scale=-1.0 / D)
    rstd1 = sc1.tile([1, SQ], F32, tag="s2", name="rstd1")
    nc.vector.reciprocal_approx_fast(rstd1[:], sd1[:])
    bneg1 = sc1.tile([1, SQ], F32, tag="s3", name="bneg1")
    nc.vector.scalar_tensor_tensor(bneg1[:], s_sb[:], -1.0 / D, rstd1[:],
                                   op0=mybir.AluOpType.mult,
                                   op1=mybir.AluOpType.mult)
    negmu_r = sc1.tile([1, SQ], BF16, tag="s4", name="negmu_r")
    nc.vector.tensor_scalar_mul(negmu_r[:], s_sb[:], -1.0 / D)
    sd_r = sc1.tile([1, SQ], BF16, tag="s5", name="sd_r")
    nc.vector.tensor_copy(sd_r[:], sd1[:])
    abc_sb = sc2.tile([P, SQ], F32, tag="sb", name="abc_sb")
    nc.gpsimd.partition_broadcast(abc_sb[:], rstd1[:])
    bbc_sb = sc2.tile([P, SQ], F32, tag="zh", name="bbc_sb")
    nc.gpsimd.partition_broadcast(bbc_sb[:], bneg1[:])

    # ---- FFN1 (bf16) ----
    hid_a = acts.tile([P, DT, SQ], BF16, tag="xtb_hi", name="hid_a")
    hid_b = acts.tile([P, DT, SQ], BF16, tag="hid_b", name="hid_b")
    for f0 in range(0, FT, 2):
        wa = wload("w1a", w1_ap[:, f0], [P, DT, P], BF16)
        wb = wload("w1b", w1_ap[:, f0 + 1], [P, DT, P], BF16)
        ps = pp.tile([P, 2, SQ], F32, tag="ps2", name="ps2")
        for k in range(DT):
            nc.tensor.matmul(ps[:, 0, :], wa[:, k, :], h18[:, k, :],
                             start=(k == 0), stop=(k == DT - 1))
            nc.tensor.matmul(ps[:, 1, :], wb[:, k, :], h18[:, k, :],
                             start=(k == 0), stop=(k == DT - 1))
        hid = hid_a if f0 < DT else hid_b
        nc.scalar.copy(hid[:, f0 % DT:f0 % DT + 2, :], ps[:])

    # real ln1 for the residual (overlaps FFN1)
    ln1 = []
    for j in range(DT):
        u = sc2.tile([P, SQ], F32, tag="u", name="u")
        nc.vector.tensor_mul(u[:], h1[j][:], abc_sb[:])
        nc.vector.tensor_add(u[:], u[:], bbc_sb[:])
        d = acts.tile([P, SQ], F32R, tag=f"ln1{j}", name=f"ln1_{j}")
        nc.scalar.activation(d[:], u[:], AF.Identity,
                             bias=consts[:, _C_BE1 + j:_C_BE1 + j + 1],
                             scale=consts[:, _C_G1 + j:_C_G1 + j + 1])
        ln1.append(d)

    # ---- FFN2 (bf16) + rank-1 LN1 fold + relu + residual + LN2 stats ----
    f2, sq2 = [], []
    ps_sum2 = pvp.tile([1, SQ], F32, tag="pv", name="ps_sum2")
    ps_sq2 = pvp.tile([1, SQ], F32, tag="pv", name="ps_sq2")
    for j in range(DT):
        w = wload("w2", w2_ap[:, j], [P, FT, P], BF16, nsplit=4)
        ps = pp.tile([P, 2, SQ], F32, tag="ps2", name="ps2")
        for f in range(FT):
            hid = hid_a if f < DT else hid_b
            nc.tensor.matmul(ps[:, 0, :], w[:, f, :], hid[:, f % DT, :],
                             start=(f == 0), stop=False)
        nc.tensor.matmul(ps[:, 0, :], fold[0:1, j * P:(j + 1) * P],
                         negmu_r[:], start=False, stop=False)
        nc.tensor.matmul(ps[:, 0, :], fold[0:1, D + j * P:D + (j + 1) * P],
                         sd_r[:], start=False, stop=True)
        if j > 0:
            colsum(ps_sum2, ps_sq2, f2[j - 1], sq2[j - 1], j - 1)
        rel = sc2.tile([P, SQ], F32R, tag="u", name="rel2")
        nc.vector.scalar_tensor_tensor(rel[:], ps[:, 0, :], 0.0, abc_sb[:],
                                       op0=mybir.AluOpType.max,
                                       op1=mybir.AluOpType.mult)
        t = acts.tile([P, SQ], F32R, tag=f"h1{j}", name=f"f2_{j}")
        nc.vector.tensor_add(t[:], rel[:], ln1[j][:])
        f2.append(t)
        sq = acts.tile([P, SQ], BF16, tag=f"sq1{j}", name=f"sq2_{j}")
        nc.scalar.activation(sq[:], t[:], AF.Square)
        sq2.append(sq)
    colsum(ps_sum2, ps_sq2, f2[DT - 1], sq2[DT - 1], DT - 1)

    # ---- LN2 chain + normalize (writes bf16, DMA per tile) ----
    s_sb2 = sc1.tile([1, SQ], F32, tag="s0", name="s_sb2")
    nc.vector.tensor_copy(s_sb2[:], ps_sum2[:])
    m22 = sc1.tile([1, SQ], F32, tag="s1", name="m22")
    nc.vector.tensor_mul(m22[:], s_sb2[:], s_sb2[:])
    a_t2 = sc1.tile([1, SQ], F32, tag="s2", name="a_t2")
    nc.vector.scalar_tensor_tensor(a_t2[:], m22[:], 1.0 / D, ps_sq2[:],
                                   op0=mybir.AluOpType.mult,
                                   op1=mybir.AluOpType.subtract)
    sd2 = sc1.tile([1, SQ], F32, tag="s1", name="sd2")
    nc.scalar.activation(sd2[:], a_t2[:], AF.Sqrt, bias=eps_t[:],
                         scale=-1.0 / D)
    rstd2 = sc1.tile([1, SQ], F32, tag="s2", name="rstd2")
    nc.vector.reciprocal_approx_fast(rstd2[:], sd2[:])
    bneg2 = sc1.tile([1, SQ], F32, tag="s3", name="bneg2")
    nc.vector.scalar_tensor_tensor(bneg2[:], s_sb2[:], -1.0 / D, rstd2[:],
                                   op0=mybir.AluOpType.mult,
                                   op1=mybir.AluOpType.mult)
    a_r = sc1.tile([1, SQ], F32R, tag="s0", name="a_r2")
    nc.vector.tensor_copy(a_r[:], rstd2[:])
    b_r = sc1.tile([1, SQ], F32R, tag="s4", name="b_r2")
    nc.vector.tensor_copy(b_r[:], bneg2[:])
    ab = pp.tile([P, 2, SQ], F32, tag="ps2", name="ab")
    nc.tensor.matmul(ab[:, 0, :], ones_row[:], a_r[:], start=True, stop=True)
    nc.tensor.matmul(ab[:, 1, :], ones_row[:], b_r[:], start=True, stop=True)
    # SBUF copies of A/B for the gpsimd-normalized tiles (gpsimd can't read
    # PSUM); the vector-normalized tiles read the PSUM broadcast directly.
    a_sb2 = sc2.tile([P, SQ], F32, tag="sb", name="a_sb2")
    nc.scalar.copy(a_sb2[:], ab[:, 0, :])
    b_sb2 = sc2.tile([P, SQ], F32, tag="zh", name="b_sb2")
    nc.scalar.copy(b_sb2[:], ab[:, 1, :])
    for j in range(DT):
        gp = j in (2, 5)
        eng = nc.gpsimd if gp else nc.vector
        u = sc2.tile([P, SQ], F32, tag="u", name="u")
        eng.tensor_mul(u[:], f2[j][:], a_sb2[:] if gp else ab[:, 0, :])
        eng.tensor_add(u[:], u[:], b_sb2[:] if gp else ab[:, 1, :])
        d = acts.tile([P, SQ], BF16, tag=f"qres{j}", name=f"y_{j}")
        nc.scalar.activation(d[:], u[:], AF.Identity,
                             bias=consts[:, _C_BE2 + j:_C_BE2 + j + 1],
                             scale=consts[:, _C_G2 + j:_C_G2 + j + 1])
        nc.sync.dma_start(yT_ap[j * P:(j + 1) * P, :], d[:])


def build():
    nc = bacc.Bacc("TRN2", target_bir_lowering=False, debug=False,
                   num_devices=NCORES)
    xT_ap = nc.dram_tensor("xT", [P, DT, S], BF16, kind="ExternalInput").ap()
    x8_ap = nc.dram_tensor("xT8", [P, DT, S], FP8, kind="ExternalInput").ap()
    wq_ap = nc.dram_tensor("Wq", [P, DT, DT, P], BF16, kind="ExternalInput").ap()
    wk_ap = nc.dram_tensor("Wk", [P, DT, DT, P], FP8, kind="ExternalInput").ap()
    wv_ap = nc.dram_tensor("Wv", [P, 2, DT, SQ], FP8, kind="ExternalInput").ap()
    wo_ap = nc.dram_tensor("Wo", [P, DT, DT, P], FP8, kind="ExternalInput").ap()
    w1_ap = nc.dram_tensor("W1", [P, FT, DT, P], BF16, kind="ExternalInput").ap()
    w2_ap = nc.dram_tensor("W2", [P, DT, FT, P], BF16, kind="ExternalInput").ap()
    consts_ap = nc.dram_tensor("consts", [P, 64], F32, kind="ExternalInput").ap()
    ones_ap = nc.dram_tensor("ones", [P, 1], F32R, kind="ExternalInput").ap()
    onesrow_ap = nc.dram_tensor("ones_row", [1, P], F32R, kind="ExternalInput").ap()
    fold_ap = nc.dram_tensor("fold", [1, 2 * D], BF16, kind="ExternalInput").ap()
    yT_ap = nc.dram_tensor("yT", [D, SQ], BF16, kind="ExternalOutput").ap()
    aps = (xT_ap, x8_ap, wq_ap, wk_ap, wv_ap, wo_ap, w1_ap, w2_ap,
           consts_ap, ones_ap, onesrow_ap, fold_ap, yT_ap)
    from contextlib import ExitStack
    with tile.TileContext(nc) as tc, ExitStack() as ctx:
        _emit(ctx, tc, aps)
    nc.compile()
    return nc


_cached_nc = None


def _get_nc():
    global _cached_nc
    if _cached_nc is None:
        _cached_nc = build()
    return _cached_nc


def _to_bf16(a):
    return np.ascontiguousarray(np.asarray(a, np.float32)).astype(
        ml_dtypes.bfloat16)


def _to_fp8(a, scale):
    return np.clip(np.asarray(a, np.float32) * scale, -240.0, 240.0).astype(
        ml_dtypes.float8_e4m3)


def _prep_in_maps(x, Wq, Wk, Wv, Wo, bo, ln1_g, ln1_b, W1, b1, W2, b2,
                  ln2_g, ln2_b):
    f = np.float32
    consts = np.zeros((P, 64), f)
    consts[:, _C_BO:_C_BO + 8] = np.asarray(bo, f).reshape(8, P).T
    consts[:, _C_B1:_C_B1 + 16] = np.asarray(b1, f).reshape(16, P).T
    consts[:, _C_B2:_C_B2 + 8] = np.asarray(b2, f).reshape(8, P).T
    consts[:, _C_G1:_C_G1 + 8] = np.asarray(ln1_g, f).reshape(8, P).T
    consts[:, _C_BE1:_C_BE1 + 8] = np.asarray(ln1_b, f).reshape(8, P).T
    consts[:, _C_G2:_C_G2 + 8] = np.asarray(ln2_g, f).reshape(8, P).T
    consts[:, _C_BE2:_C_BE2 + 8] = np.asarray(ln2_b, f).reshape(8, P).T
    ones = np.ones((P, 1), f)
    ones_row = np.ones((1, P), f)
    W1f = np.asarray(W1, np.float64)
    W2f = np.asarray(W2, np.float64)
    g1v = np.asarray(ln1_g, np.float64)
    b1v = np.asarray(ln1_b, np.float64)
    c1 = np.asarray(b1, np.float64) + (b1v[:, None] * W1f).sum(axis=0)
    W1g = (g1v[:, None] * W1f).astype(f)
    w2g1 = (g1v[:, None] * W1f).sum(axis=0) @ W2f
    c2 = np.asarray(b2, np.float64) + c1 @ W2f
    fold = np.concatenate([w2g1, c2]).astype(f)[None, :]

    def pack_st(W, dtype_fn):
        # [D_in, N] -> [P, N/P, D_in/P, P] stationary tiles
        din, n = W.shape
        return np.ascontiguousarray(
            dtype_fn(np.asarray(W, f).reshape(din // P, P, n // P, P)
                     .transpose(1, 2, 0, 3)))

    shared = {
        "Wq": pack_st(np.asarray(Wq, f), _to_bf16),
        "Wk": pack_st(np.asarray(Wk, f), lambda a: _to_fp8(a, ALPHA)),
        "Wo": pack_st(np.asarray(Wo, f), lambda a: _to_fp8(a, ALPHA)),
        "W1": pack_st(W1g, _to_bf16),
        "W2": pack_st(np.asarray(W2, f), _to_bf16),
        "Wv": np.ascontiguousarray(
            _to_fp8(np.asarray(Wv, f).reshape(DT, P, 2, SQ)
                    .transpose(1, 2, 0, 3), ALPHA)),
        "consts": consts, "ones": ones, "ones_row": ones_row,
        "fold": _to_bf16(fold),
    }
    xt = np.asarray(x, f).transpose(0, 2, 1)  # [B, D, S]
    in_maps = []
    for core in range(NCORES):
        b, off = core // 2, (core % 2) * SQ
        if off == 0:
            xrot = xt[b]
        else:
            xrot = np.concatenate([xt[b][:, off:], xt[b][:, :off]], axis=1)
        xr = xrot.reshape(DT, P, S).transpose(1, 0, 2)
        in_maps.append(dict(shared, xT=np.ascontiguousarray(_to_bf16(xr)),
                            xT8=np.ascontiguousarray(_to_fp8(xr, 1.0))))
    return in_maps


def run(inputs, trace=False, tmpdir=None):
    """Run the kernel on 8 cores. Returns (y, BassKernelResults)."""
    nc = _get_nc()
    in_maps = _prep_in_maps(
        inputs["x"], inputs["Wq"], inputs["Wk"], inputs["Wv"], inputs["Wo"],
        inputs["bo"], inputs["ln1_g"], inputs["ln1_b"], inputs["W1"],
        inputs["b1"], inputs["W2"], inputs["b2"], inputs["ln2_g"],
        inputs["ln2_b"])
    try:
        res = bass_utils.run_bass_kernel_spmd(nc, in_maps, list(range(NCORES)),
                                              trace=trace, tmpdir=tmpdir)
    except Exception:
        import time as _time
        _time.sleep(2.0)
        res = bass_utils.run_bass_kernel_spmd(nc, in_maps, list(range(NCORES)),
                                              trace=trace, tmpdir=tmpdir)
    y = np.empty((B, S, D), np.float32)
    for core in range(NCORES):
        b, off = core // 2, (core % 2) * SQ
        y[b, off:off + SQ, :] = res.results[core]["yT"].astype(np.float32).T
    return y, res


def kernel(x, mask, Wq, Wk, Wv, Wo, bo, ln1_g, ln1_b, W1, b1, W2, b2,
           ln2_g, ln2_b):
    # mask is all-ones per the problem spec -> identity in the reference.
    y, _ = run(dict(x=x, Wq=Wq, Wk=Wk, Wv=Wv, Wo=Wo, bo=bo, ln1_g=ln1_g,
                    ln1_b=ln1_b, W1=W1, b1=b1, W2=W2, b2=b2, ln2_g=ln2_g,
                    ln2_b=ln2_b))
    return y


# revision 14
# speedup vs baseline: 1.2758x; 1.0622x over previous
"""Trainium2 Bass kernel for nn_EncoderLayer (B=4, S=1024, D=1024, H=16, FF=2048).

Sharding: 8 cores = 4 batches x 2 sequence-halves, each core redundantly
computes K/V for its whole batch and runs the layer for its own 512 query
rows. Odd cores see the sequence rotated by 512 (softmax over keys is
permutation-invariant).

Mixed-precision PE pipeline:
  - K/V projections, QK^T scores, exp, attn*V and the output projection run
    in fp8e4 (TRN E4M3) with MatmulPerfMode.DoubleRow (two 128-row
    contraction tiles per instruction = 2x PE throughput). fp8 weights are
    pre-scaled by 32 on the host; the 1/32 is folded into PSUM->SBUF copies.
  - Q projection and both FFN matmuls stay bf16: their error lands on the
    residual stream where fp8 blows the 2e-2 budget (measured 5e-2 all-fp8
    vs 3.2e-3 with these three in bf16).

Schedule: the softmax exp stream (~57us on the scalar engine) is the
second-longest pole after the PE, so scores for head-pair j are emitted
immediately after the K projection of tile j, with attn*V lagging one pair
(PV(j-1) fills the PE while the vector engine re-quantizes k8[j]). V(c=0)
is interleaved into the j=0 block, V(c=1) into the j=4 block (attn*V for
pairs 0-3 only reads heads 0-7 = the c=0 half of V).

Layernorm plumbing (feature-major activations, stats via ones-column
matmuls, gamma/beta folded into W1 + rank-1 W2 corrections) as in v1; the
LN column-sum matmuls are interleaved (lag-one) into the Wo/FFN2 loops and
the LN2 normalize reads the A/B broadcast PSUM directly and writes bf16.
"""

import sys
import types

import numpy as np
import ml_dtypes


def _shim_axon_hooks():
    try:
        import antenv.axon_hooks  # noqa: F401
    except Exception:
        mod = types.ModuleType("antenv.axon_hooks")
        mod.get_axon_ntff_profile_hook = lambda: None
        mod.set_axon_ntff_profile_hook = lambda h: None
        sys.modules["antenv.axon_hooks"] = mod


_shim_axon_hooks()

from concourse import bacc, mybir, tile  # noqa: E402
from concourse import bass_utils  # noqa: E402

F32 = mybir.dt.float32
F32R = mybir.dt.float32r
BF16 = mybir.dt.bfloat16
FP8 = mybir.dt.float8e4
AF = mybir.ActivationFunctionType
DR = mybir.MatmulPerfMode.DoubleRow

B, S, D, H, DH, FF = 4, 1024, 1024, 16, 64, 2048
SQ = 512
P = 128
DT = D // P
FT = FF // P
ST = S // P
NCORES = 8
EPS = 1e-6
SCALE = 1.0 / 32.0
ALPHA = 32.0

_C_BO = 0
_C_B1 = 8
_C_B2 = 24
_C_G1 = 32
_C_BE1 = 40
_C_G2 = 48
_C_BE2 = 56


def _emit(ctx, tc, aps):
    nc = tc.nc
    (xT_ap, x8_ap, wq_ap, wk_ap, wv_ap, wo_ap, w1_ap, w2_ap, consts_ap,
     ones_ap, onesrow_ap, fold_ap, yT_ap) = aps

    acts = ctx.enter_context(tc.tile_pool(name="acts", bufs=1))
    wts = ctx.enter_context(tc.tile_pool(name="wts", bufs=2))
    e2p = ctx.enter_context(tc.tile_pool(name="e2p", bufs=8))
    sc2 = ctx.enter_context(tc.tile_pool(name="sc2", bufs=2))
    sc1 = ctx.enter_context(tc.tile_pool(name="sc1", bufs=1))
    pp = ctx.enter_context(tc.tile_pool(name="pp", bufs=1, space="PSUM"))
    pss = ctx.enter_context(tc.tile_pool(name="pss", bufs=2, space="PSUM"))
    pvp = ctx.enter_context(tc.tile_pool(name="pvp", bufs=2, space="PSUM"))

    def wload(tag, src_ap, shape, dtype, nsplit=2):
        w = wts.tile(shape, dtype, tag=tag, name=tag)
        step = shape[1] // nsplit
        for q in range(nsplit):
            nc.sync.dma_start(w[:, q * step:(q + 1) * step, :],
                              src_ap[:, q * step:(q + 1) * step, :])
        return w

    # ---- input DMA: xt8 + wk0 first (K0 gates the exp stream), then Q's ----
    xt8 = acts.tile([P, DT, S], FP8, tag="xt8", name="xt8")
    for q in range(4):
        nc.sync.dma_start(xt8[:, 2 * q:2 * q + 2, :],
                          x8_ap[:, 2 * q:2 * q + 2, :])
    wk0 = wload("wka", wk_ap[:, 0], [P, DT, P], FP8)
    wqa = wload("wqa", wq_ap[:, 0], [P, DT, P], BF16)
    wqb = wload("wqb", wq_ap[:, 1], [P, DT, P], BF16)
    xtb_a = acts.tile([P, 4, SQ], BF16, tag="xtb_a", name="xtb_a")
    nc.sync.dma_start(xtb_a[:, 0:2, :], xT_ap[:, 0:2, 0:SQ])
    nc.sync.dma_start(xtb_a[:, 2:4, :], xT_ap[:, 2:4, 0:SQ])
    xtb_b = acts.tile([P, 4, SQ], BF16, tag="xtb_b", name="xtb_b")
    nc.sync.dma_start(xtb_b[:, 0:2, :], xT_ap[:, 4:6, 0:SQ])
    nc.sync.dma_start(xtb_b[:, 2:4, :], xT_ap[:, 6:8, 0:SQ])
    consts = acts.tile([P, 64], F32, tag="consts", name="consts")
    nc.sync.dma_start(consts[:], consts_ap[:])
    ones_r = acts.tile([P, 1], F32R, tag="ones", name="ones")
    nc.sync.dma_start(ones_r[:], ones_ap[:])
    ones_row = acts.tile([1, P], F32R, tag="ones_row", name="ones_row")
    nc.sync.dma_start(ones_row[:], onesrow_ap[:])
    ones_b = acts.tile([P, 1], BF16, tag="ones_b", name="ones_b")
    nc.vector.memset(ones_b[:], 1.0)
    fold = acts.tile([1, 2 * D], BF16, tag="fold", name="fold")
    nc.sync.dma_start(fold[:], fold_ap[:])

    def xlo(k):
        return xtb_a[:, k, :] if k < 4 else xtb_b[:, k - 4, :]

    # ---- attention machinery ----
    q_res = []
    q8 = acts.tile([P, DT, SQ], FP8, tag="q8", name="q8")
    vr8 = acts.tile([P, ST, H, DH + 1], FP8, tag="vr8", name="vr8")
    nc.vector.memset(vr8[:, :, :, DH:DH + 1], 1.0)
    attn8 = acts.tile([P, DT, SQ], FP8, tag="attn8", name="attn8")
    k8 = [None] * DT
    e2s = {}
    pvs = {}

    def qpair(j0, wa, wb):
        ps = pp.tile([P, 2, SQ], F32, tag="ps2", name="ps2")
        for k in range(DT):
            nc.tensor.matmul(ps[:, 0, :], wa[:, k, :], xlo(k),
                             start=(k == 0), stop=(k == DT - 1))
            nc.tensor.matmul(ps[:, 1, :], wb[:, k, :], xlo(k),
                             start=(k == 0), stop=(k == DT - 1))
        for h in range(2):
            j = j0 + h
            qr = acts.tile([P, SQ], F32R, tag=f"qres{j}", name=f"qres{j}")
            nc.scalar.copy(qr[:], ps[:, h, :])
            q_res.append(qr)
            nc.vector.tensor_copy(q8[:, j, :], ps[:, h, :])

    def kproj(j, w):
        ps = pp.tile([P, 2, SQ], F32, tag="ps2", name="ps2")
        for k in range(0, DT, 2):
            nc.tensor.matmul(ps[:, 0, :], w[:, k:k + 2, :],
                             xt8[:, k:k + 2, 0:SQ],
                             start=(k == 0), stop=(k == DT - 2), perf_mode=DR)
            nc.tensor.matmul(ps[:, 1, :], w[:, k:k + 2, :],
                             xt8[:, k:k + 2, SQ:S],
                             start=(k == 0), stop=(k == DT - 2), perf_mode=DR)
        kj = acts.tile([P, S], FP8, tag=f"k8{j}", name=f"k8{j}")
        nc.vector.tensor_scalar_mul(
            kj[:].rearrange("p (c q) -> p c q", c=2), ps[:], 1.0 / ALPHA)
        k8[j] = kj

    def v_pair(c, st0, wv):
        """V projection for key tiles st0, st0+1 (one psum tile, one copy)."""
        ps = pp.tile([P, 2, SQ], F32, tag="ps2", name="ps2")
        for si in range(2):
            for k in range(0, DT, 2):
                nc.tensor.matmul(ps[:, si, :],
                                 xt8[:, k:k + 2, (st0 + si) * P:(st0 + si + 1) * P],
                                 wv[:, k:k + 2, :],
                                 start=(k == 0), stop=(k == DT - 2),
                                 perf_mode=DR)
        nc.vector.tensor_scalar_mul(
            vr8[:, st0:st0 + 2, c * 8:(c + 1) * 8, 0:DH],
            ps[:].rearrange("p s (h d) -> p s h d", d=DH), 1.0 / ALPHA)

    def scx(j, st2):
        """scores + exp for head pair (2j, 2j+1), key tiles 2*st2, 2*st2+1."""
        e2 = e2p.tile([P, 2, 2, SQ], FP8, tag="e2", name="e2")
        for sti in range(2):
            st = 2 * st2 + sti
            sl = slice(st * P, (st + 1) * P)
            ps = pss.tile([P, 2, SQ], F32, tag="pss", name="pss")
            nc.tensor.matmul(ps[:, 0, :], k8[j][0:DH, sl], q8[0:DH, j, :],
                             start=True, stop=True)
            nc.tensor.matmul(ps[:, 1, :], k8[j][DH:P, sl], q8[DH:P, j, :],
                             start=True, stop=True)
            nc.scalar.activation(e2[:, sti, :, :], ps[:], AF.Exp, scale=SCALE)
        e2s[(j, st2)] = e2

    def pv_acc(j, st2):
        if st2 == 0:
            pvs[j] = (pvp.tile([DH + 1, SQ], F32, tag="pv", name="pv"),
                      pvp.tile([DH + 1, SQ], F32, tag="pv", name="pv"))
        pv0, pv1 = pvs[j]
        e2 = e2s.pop((j, st2))
        nc.tensor.matmul(pv0[:], vr8[:, 2 * st2:2 * st2 + 2, 2 * j, :],
                         e2[:, :, 0, :], start=(st2 == 0),
                         stop=(st2 == ST // 2 - 1), perf_mode=DR)
        nc.tensor.matmul(pv1[:], vr8[:, 2 * st2:2 * st2 + 2, 2 * j + 1, :],
                         e2[:, :, 1, :], start=(st2 == 0),
                         stop=(st2 == ST // 2 - 1), perf_mode=DR)

    def pv_all(j):
        for st2 in range(4):
            pv_acc(j, st2)

    def norm(j):
        pv0, pv1 = pvs.pop(j)
        for half, pv in ((0, pv0), (1, pv1)):
            rows = slice(half * DH, half * DH + DH)
            zh = sc2.tile([1, SQ], F32, tag="zh", name="zh")
            nc.vector.tensor_copy(zh[:], pv[DH:DH + 1, :])
            iz = sc2.tile([1, SQ], F32, tag="zh", name="iz")
            nc.vector.reciprocal_approx_fast(iz[:], zh[:])
            bz = sc2.tile([DH, SQ], F32, tag="sb", name="sb")
            nc.gpsimd.partition_broadcast(bz[:], iz[:])
            nc.vector.tensor_mul(attn8[rows, j, :], pv[0:DH, :], bz[:])

    # ---- block schedule: K(j) first so exp starts ~9us in; Q pairs and V
    # groups fill the PE between the exp-gated scores/PV chains. V(c=0)
    # completes by block 2 (pv pairs 0-3 read heads 0-7), V(c=1) by block 5.
    # attn*V for pair p runs 1-2 blocks after its exps; e2 pool depth 8
    # covers the two in-flight pairs. ----
    wv0 = wload("wv", wv_ap[:, 0], [P, DT, SQ], FP8, nsplit=2)
    kproj(0, wk0)
    qpair(0, wqa, wqb)
    scx(0, 0)
    v_pair(0, 0, wv0)
    scx(0, 1)
    scx(0, 2)
    scx(0, 3)
    wv1 = None
    for j in range(1, DT):
        w = wload("wka" if j % 2 == 0 else "wkb", wk_ap[:, j], [P, DT, P],
                  FP8, nsplit=1)
        kproj(j, w)
        if j == 3:
            wv1 = wload("wv", wv_ap[:, 1], [P, DT, SQ], FP8, nsplit=2)
        # V pairs for this block: c0 done by block 2, c1 by block 5
        vg = {1: [(0, 2), (0, 4)], 2: [(0, 6)], 3: [(1, 0)], 4: [(1, 2)],
              5: [(1, 4), (1, 6)]}.get(j, [])
        scx(j, 0)
        if vg:
            v_pair(vg[0][0], vg[0][1], wv0 if vg[0][0] == 0 else wv1)
        scx(j, 1)
        if j < 4:
            qpair(2 * j,
                  wload("wqa", wq_ap[:, 2 * j], [P, DT, P], BF16, nsplit=1),
                  wload("wqb", wq_ap[:, 2 * j + 1], [P, DT, P], BF16,
                        nsplit=1))
        scx(j, 2)
        if len(vg) > 1:
            v_pair(vg[1][0], vg[1][1], wv0 if vg[1][0] == 0 else wv1)
        if j >= 2:
            pv_all(j - 2)
            norm(j - 2)
        scx(j, 3)
    for j in range(DT - 2, DT):
        pv_all(j)
        norm(j)

    # ---- output projection (fp8 DoubleRow) + relu + residual + LN1 stats ----
    h1, sq1 = [], []
    h18 = acts.tile([P, DT, SQ], BF16, tag="h18", name="h18")
    ps_sum1 = pvp.tile([1, SQ], F32, tag="pv", name="ps_sum1")
    ps_sq1 = pvp.tile([1, SQ], F32, tag="pv", name="ps_sq1")

    def colsum(ps_sum, ps_sq, src, sq, j):
        nc.tensor.matmul(ps_sum[:], ones_r[:], src[:],
                         start=(j == 0), stop=(j == DT - 1))
        nc.tensor.matmul(ps_sq[:], ones_b[:], sq[:],
                         start=(j == 0), stop=(j == DT - 1))

    for j0 in range(0, DT, 2):
        wa = wload("woa", wo_ap[:, j0], [P, DT, P], FP8, nsplit=1)
        wb = wload("wob", wo_ap[:, j0 + 1], [P, DT, P], FP8, nsplit=1)
        ps = pss.tile([P, 2, SQ], F32, tag="pss", name="ps_wo")
        for k in range(0, DT, 2):
            nc.tensor.matmul(ps[:, 0, :], wa[:, k:k + 2, :],
                             attn8[:, k:k + 2, :],
                             start=(k == 0), stop=(k == DT - 2), perf_mode=DR)
            nc.tensor.matmul(ps[:, 1, :], wb[:, k:k + 2, :],
                             attn8[:, k:k + 2, :],
                             start=(k == 0), stop=(k == DT - 2), perf_mode=DR)
        # lag-one interleave of the LN1 column sums
        for j in (j0 - 2, j0 - 1):
            if j >= 0:
                colsum(ps_sum1, ps_sq1, h1[j], sq1[j], j)
        for h in range(2):
            j = j0 + h
            rel = sc2.tile([P, SQ], F32R, tag="u", name="rel")
            nc.scalar.activation(rel[:], ps[:, h, :], AF.Relu,
                                 bias=consts[:, _C_BO + j:_C_BO + j + 1],
                                 scale=1.0 / ALPHA)
            t = acts.tile([P, SQ], F32R, tag=f"h1{j}", name=f"h1_{j}")
            nc.vector.tensor_add(t[:], rel[:], q_res[j][:])
            h1.append(t)
            nc.scalar.copy(h18[:, j, :], t[:])
            sq = acts.tile([P, SQ], BF16, tag=f"sq1{j}", name=f"sq1_{j}")
            nc.vector.tensor_mul(sq[:], t[:], t[:])
            sq1.append(sq)
    for j in (DT - 2, DT - 1):
        colsum(ps_sum1, ps_sq1, h1[j], sq1[j], j)

    # LN1 chain (gamma/beta folded into W1 / rank-1 W2 fold)
    s_sb = sc1.tile([1, SQ], F32, tag="s0", name="s_sb")
    nc.vector.tensor_copy(s_sb[:], ps_sum1[:])
    m2 = sc1.tile([1, SQ], F32, tag="s1", name="m2")
    nc.vector.tensor_mul(m2[:], s_sb[:], s_sb[:])
    a_t = sc1.tile([1, SQ], F32, tag="s2", name="a_t")
    nc.vector.scalar_tensor_tensor(a_t[:], m2[:], 1.0 / D, ps_sq1[:],
                                   op0=mybir.AluOpType.mult,
                                   op1=mybir.AluOpType.subtract)
    eps_t = sc1.tile([1, 1], F32, tag="eps", name="eps")
    nc.vector.memset(eps_t[:], EPS)
    sd1 = sc1.tile([1, SQ], F32, tag="s1", name="sd1")
    nc.scalar.activation(sd1[:], a_t[:], AF.Sqrt, bias=eps_t[:], scale=-1.0 / D)
    rstd1 = sc1.tile([1, SQ], F32, tag="s2", name="rstd1")
    nc.vector.reciprocal_approx_fast(rstd1[:], sd1[:])
    bneg1 = sc1.tile([1, SQ], F32, tag="s3", name="bneg1")
    nc.vector.scalar_tensor_tensor(bneg1[:], s_sb[:], -1.0 / D, rstd1[:],
                                   op0=mybir.AluOpType.mult,
                                   op1=mybir.AluOpType.mult)
    negmu_r = sc1.tile([1, SQ], BF16, tag="s4", name="negmu_r")
    nc.vector.tensor_scalar_mul(negmu_r[:], s_sb[:], -1.0 / D)
    sd_r = sc1.tile([1, SQ], BF16, tag="s5", name="sd_r")
    nc.vector.tensor_copy(sd_r[:], sd1[:])
    abc_sb = sc2.tile([P, SQ], F32, tag="sb", name="abc_sb")
    nc.gpsimd.partition_broadcast(abc_sb[:], rstd1[:])
    bbc_sb = sc2.tile([P, SQ], F32, tag="zh", name="bbc_sb")
    nc.gpsimd.partition_broadcast(bbc_sb[:], bneg1[:])

    # ---- FFN1 (bf16) ----
    hid_a = acts.tile([P, DT, SQ], BF16, tag="xtb_hi", name="hid_a")
    hid_b = acts.tile([P, DT, SQ], BF16, tag="hid_b", name="hid_b")
    for f0 in range(0, FT, 2):
        wa = wload("w1a", w1_ap[:, f0], [P, DT, P], BF16)
        wb = wload("w1b", w1_ap[:, f0 + 1], [P, DT, P], BF16)
        ps = pss.tile([P, 2, SQ], F32, tag="pss", name="ps_f1")
        for k in range(DT):
            nc.tensor.matmul(ps[:, 0, :], wa[:, k, :], h18[:, k, :],
                             start=(k == 0), stop=(k == DT - 1))
            nc.tensor.matmul(ps[:, 1, :], wb[:, k, :], h18[:, k, :],
                             start=(k == 0), stop=(k == DT - 1))
        hid = hid_a if f0 < DT else hid_b
        nc.scalar.copy(hid[:, f0 % DT:f0 % DT + 2, :], ps[:])

    # real ln1 for the residual (overlaps FFN1)
    ln1 = []
    for j in range(DT):
        u = sc2.tile([P, SQ], F32, tag="u", name="u")
        nc.vector.tensor_mul(u[:], h1[j][:], abc_sb[:])
        nc.vector.tensor_add(u[:], u[:], bbc_sb[:])
        d = acts.tile([P, SQ], F32R, tag=f"ln1{j}", name=f"ln1_{j}")
        nc.scalar.activation(d[:], u[:], AF.Identity,
                             bias=consts[:, _C_BE1 + j:_C_BE1 + j + 1],
                             scale=consts[:, _C_G1 + j:_C_G1 + j + 1])
        ln1.append(d)

    # ---- FFN2 (bf16) + rank-1 LN1 fold + relu + residual + LN2 stats ----
    f2, sq2 = [], []
    ps_sum2 = pvp.tile([1, SQ], F32, tag="pv", name="ps_sum2")
    ps_sq2 = pvp.tile([1, SQ], F32, tag="pv", name="ps_sq2")
    for j in range(DT):
        w = wload("w2", w2_ap[:, j], [P, FT, P], BF16, nsplit=4)
        ps = pss.tile([P, 2, SQ], F32, tag="pss", name="ps_f2")
        for f in range(FT):
            hid = hid_a if f < DT else hid_b
            nc.tensor.matmul(ps[:, 0, :], w[:, f, :], hid[:, f % DT, :],
                             start=(f == 0), stop=False)
        nc.tensor.matmul(ps[:, 0, :], fold[0:1, j * P:(j + 1) * P],
                         negmu_r[:], start=False, stop=False)
        nc.tensor.matmul(ps[:, 0, :], fold[0:1, D + j * P:D + (j + 1) * P],
                         sd_r[:], start=False, stop=True)
        if j > 0:
            colsum(ps_sum2, ps_sq2, f2[j - 1], sq2[j - 1], j - 1)
        rel = sc2.tile([P, SQ], F32R, tag="u", name="rel2")
        nc.vector.scalar_tensor_tensor(rel[:], ps[:, 0, :], 0.0, abc_sb[:],
                                       op0=mybir.AluOpType.max,
                                       op1=mybir.AluOpType.mult)
        t = acts.tile([P, SQ], F32R, tag=f"h1{j}", name=f"f2_{j}")
        nc.vector.tensor_add(t[:], rel[:], ln1[j][:])
        f2.append(t)
        sq = acts.tile([P, SQ], BF16, tag=f"sq1{j}", name=f"sq2_{j}")
        nc.scalar.activation(sq[:], t[:], AF.Square)
        sq2.append(sq)
    colsum(ps_sum2, ps_sq2, f2[DT - 1], sq2[DT - 1], DT - 1)

    # ---- LN2 chain + normalize (writes bf16, DMA per tile) ----
    s_sb2 = sc1.tile([1, SQ], F32, tag="s0", name="s_sb2")
    nc.vector.tensor_copy(s_sb2[:], ps_sum2[:])
    m22 = sc1.tile([1, SQ], F32, tag="s1", name="m22")
    nc.vector.tensor_mul(m22[:], s_sb2[:], s_sb2[:])
    a_t2 = sc1.tile([1, SQ], F32, tag="s2", name="a_t2")
    nc.vector.scalar_tensor_tensor(a_t2[:], m22[:], 1.0 / D, ps_sq2[:],
                                   op0=mybir.AluOpType.mult,
                                   op1=mybir.AluOpType.subtract)
    sd2 = sc1.tile([1, SQ], F32, tag="s1", name="sd2")
    nc.scalar.activation(sd2[:], a_t2[:], AF.Sqrt, bias=eps_t[:],
                         scale=-1.0 / D)
    rstd2 = sc1.tile([1, SQ], F32, tag="s2", name="rstd2")
    nc.vector.reciprocal_approx_fast(rstd2[:], sd2[:])
    bneg2 = sc1.tile([1, SQ], F32, tag="s3", name="bneg2")
    nc.vector.scalar_tensor_tensor(bneg2[:], s_sb2[:], -1.0 / D, rstd2[:],
                                   op0=mybir.AluOpType.mult,
                                   op1=mybir.AluOpType.mult)
    a_r = sc1.tile([1, SQ], F32R, tag="s0", name="a_r2")
    nc.vector.tensor_copy(a_r[:], rstd2[:])
    b_r = sc1.tile([1, SQ], F32R, tag="s4", name="b_r2")
    nc.vector.tensor_copy(b_r[:], bneg2[:])
    ab = pss.tile([P, 2, SQ], F32, tag="pss", name="ab")
    nc.tensor.matmul(ab[:, 0, :], ones_row[:], a_r[:], start=True, stop=True)
    nc.tensor.matmul(ab[:, 1, :], ones_row[:], b_r[:], start=True, stop=True)
    # SBUF copies of A/B for the gpsimd-normalized tiles (gpsimd can't read
    # PSUM); the vector-normalized tiles read the PSUM broadcast directly.
    a_sb2 = sc2.tile([P, SQ], F32, tag="sb", name="a_sb2")
    nc.scalar.copy(a_sb2[:], ab[:, 0, :])
    b_sb2 = sc2.tile([P, SQ], F32, tag="zh", name="b_sb2")
    nc.scalar.copy(b_sb2[:], ab[:, 1, :])
    for j in range(DT):
        gp = j in (2, 5)
        eng = nc.gpsimd if gp else nc.vector
        u = sc2.tile([P, SQ], F32, tag="u", name="u")
        eng.tensor_mul(u[:], f2[j][:], a_sb2[:] if gp else ab[:, 0, :])
        eng.tensor_add(u[:], u[:], b_sb2[:] if gp else ab[:, 1, :])
        d = acts.tile([P, SQ], BF16, tag=f"qres{j}", name=f"y_{j}")
        nc.scalar.activation(d[:], u[:], AF.Identity,
                             bias=consts[:, _C_BE2 + j:_C_BE2 + j + 1],
                             scale=consts[:, _C_G2 + j:_C_G2 + j + 1])
        nc.sync.dma_start(yT_ap[j * P:(j + 1) * P, :], d[:])


def build():
    nc = bacc.Bacc("TRN2", target_bir_lowering=False, debug=False,
                   num_devices=NCORES)
    xT_ap = nc.dram_tensor("xT", [P, DT, S], BF16, kind="ExternalInput").ap()
    x8_ap = nc.dram_tensor("xT8", [P, DT, S], FP8, kind="ExternalInput").ap()
    wq_ap = nc.dram_tensor("Wq", [P, DT, DT, P], BF16, kind="ExternalInput").ap()
    wk_ap = nc.dram_tensor("Wk", [P, DT, DT, P], FP8, kind="ExternalInput").ap()
    wv_ap = nc.dram_tensor("Wv", [P, 2, DT, SQ], FP8, kind="ExternalInput").ap()
    wo_ap = nc.dram_tensor("Wo", [P, DT, DT, P], FP8, kind="ExternalInput").ap()
    w1_ap = nc.dram_tensor("W1", [P, FT, DT, P], BF16, kind="ExternalInput").ap()
    w2_ap = nc.dram_tensor("W2", [P, DT, FT, P], BF16, kind="ExternalInput").ap()
    consts_ap = nc.dram_tensor("consts", [P, 64], F32, kind="ExternalInput").ap()
    ones_ap = nc.dram_tensor("ones", [P, 1], F32R, kind="ExternalInput").ap()
    onesrow_ap = nc.dram_tensor("ones_row", [1, P], F32R, kind="ExternalInput").ap()
    fold_ap = nc.dram_tensor("fold", [1, 2 * D], BF16, kind="ExternalInput").ap()
    yT_ap = nc.dram_tensor("yT", [D, SQ], BF16, kind="ExternalOutput").ap()
    aps = (xT_ap, x8_ap, wq_ap, wk_ap, wv_ap, wo_ap, w1_ap, w2_ap,
           consts_ap, ones_ap, onesrow_ap, fold_ap, yT_ap)
    from contextlib import ExitStack
    with tile.TileContext(nc) as tc, ExitStack() as ctx:
        _emit(ctx, tc, aps)
    nc.compile()
    return nc


_cached_nc = None


def _get_nc():
    global _cached_nc
    if _cached_nc is None:
        _cached_nc = build()
    return _cached_nc


def _to_bf16(a):
    return np.ascontiguousarray(np.asarray(a, np.float32)).astype(
        ml_dtypes.bfloat16)


def _to_fp8(a, scale):
    return np.clip(np.asarray(a, np.float32) * scale, -240.0, 240.0).astype(
        ml_dtypes.float8_e4m3)


def _prep_in_maps(x, Wq, Wk, Wv, Wo, bo, ln1_g, ln1_b, W1, b1, W2, b2,
                  ln2_g, ln2_b):
    f = np.float32
    consts = np.zeros((P, 64), f)
    consts[:, _C_BO:_C_BO + 8] = np.asarray(bo, f).reshape(8, P).T
    consts[:, _C_B1:_C_B1 + 16] = np.asarray(b1, f).reshape(16, P).T
    consts[:, _C_B2:_C_B2 + 8] = np.asarray(b2, f).reshape(8, P).T
    consts[:, _C_G1:_C_G1 + 8] = np.asarray(ln1_g, f).reshape(8, P).T
    consts[:, _C_BE1:_C_BE1 + 8] = np.asarray(ln1_b, f).reshape(8, P).T
    consts[:, _C_G2:_C_G2 + 8] = np.asarray(ln2_g, f).reshape(8, P).T
    consts[:, _C_BE2:_C_BE2 + 8] = np.asarray(ln2_b, f).reshape(8, P).T
    ones = np.ones((P, 1), f)
    ones_row = np.ones((1, P), f)
    W1f = np.asarray(W1, np.float64)
    W2f = np.asarray(W2, np.float64)
    g1v = np.asarray(ln1_g, np.float64)
    b1v = np.asarray(ln1_b, np.float64)
    c1 = np.asarray(b1, np.float64) + (b1v[:, None] * W1f).sum(axis=0)
    W1g = (g1v[:, None] * W1f).astype(f)
    w2g1 = (g1v[:, None] * W1f).sum(axis=0) @ W2f
    c2 = np.asarray(b2, np.float64) + c1 @ W2f
    fold = np.concatenate([w2g1, c2]).astype(f)[None, :]

    def pack_st(W, dtype_fn):
        # [D_in, N] -> [P, N/P, D_in/P, P] stationary tiles
        din, n = W.shape
        return np.ascontiguousarray(
            dtype_fn(np.asarray(W, f).reshape(din // P, P, n // P, P)
                     .transpose(1, 2, 0, 3)))

    shared = {
        "Wq": pack_st(np.asarray(Wq, f), _to_bf16),
        "Wk": pack_st(np.asarray(Wk, f), lambda a: _to_fp8(a, ALPHA)),
        "Wo": pack_st(np.asarray(Wo, f), lambda a: _to_fp8(a, ALPHA)),
        "W1": pack_st(W1g, _to_bf16),
        "W2": pack_st(np.asarray(W2, f), _to_bf16),
        "Wv": np.ascontiguousarray(
            _to_fp8(np.asarray(Wv, f).reshape(DT, P, 2, SQ)
                    .transpose(1, 2, 0, 3), ALPHA)),
        "consts": consts, "ones": ones, "ones_row": ones_row,
        "fold": _to_bf16(fold),
    }
    xt = np.asarray(x, f).transpose(0, 2, 1)  # [B, D, S]
    in_maps = []
    for core in range(NCORES):
        b, off = core // 2, (core % 2) * SQ
        if off == 0:
            xrot = xt[b]
        else:
            xrot = np.concatenate([xt[b][:, off:], xt[b][:, :off]], axis=1)
        xr = xrot.reshape(DT, P, S).transpose(1, 0, 2)
        in_maps.append(dict(shared, xT=np.ascontiguousarray(_to_bf16(xr)),
                            xT8=np.ascontiguousarray(_to_fp8(xr, 1.0))))
    return in_maps


def run(inputs, trace=False, tmpdir=None):
    """Run the kernel on 8 cores. Returns (y, BassKernelResults)."""
    nc = _get_nc()
    in_maps = _prep_in_maps(
        inputs["x"], inputs["Wq"], inputs["Wk"], inputs["Wv"], inputs["Wo"],
        inputs["bo"], inputs["ln1_g"], inputs["ln1_b"], inputs["W1"],
        inputs["b1"], inputs["W2"], inputs["b2"], inputs["ln2_g"],
        inputs["ln2_b"])
    try:
        res = bass_utils.run_bass_kernel_spmd(nc, in_maps, list(range(NCORES)),
                                              trace=trace, tmpdir=tmpdir)
    except Exception:
        import time as _time
        _time.sleep(2.0)
        res = bass_utils.run_bass_kernel_spmd(nc, in_maps, list(range(NCORES)),
                                              trace=trace, tmpdir=tmpdir)
    y = np.empty((B, S, D), np.float32)
    for core in range(NCORES):
        b, off = core // 2, (core % 2) * SQ
        y[b, off:off + SQ, :] = res.results[core]["yT"].astype(np.float32).T
    return y, res


def kernel(x, mask, Wq, Wk, Wv, Wo, bo, ln1_g, ln1_b, W1, b1, W2, b2,
           ln2_g, ln2_b):
    # mask is all-ones per the problem spec -> identity in the reference.
    y, _ = run(dict(x=x, Wq=Wq, Wk=Wk, Wv=Wv, Wo=Wo, bo=bo, ln1_g=ln1_g,
                    ln1_b=ln1_b, W1=W1, b1=b1, W2=W2, b2=b2, ln2_g=ln2_g,
                    ln2_b=ln2_b))
    return y


# revision 16
# speedup vs baseline: 1.2892x; 1.0105x over previous
"""Trainium2 Bass kernel for nn_EncoderLayer (B=4, S=1024, D=1024, H=16, FF=2048).

Sharding: 8 cores = 4 batches x 2 sequence-halves, each core redundantly
computes K/V for its whole batch and runs the layer for its own 512 query
rows. Odd cores see the sequence rotated by 512 (softmax over keys is
permutation-invariant).

Mixed-precision PE pipeline:
  - K/V projections, QK^T scores, exp, attn*V and the output projection run
    in fp8e4 (TRN E4M3) with MatmulPerfMode.DoubleRow (two 128-row
    contraction tiles per instruction = 2x PE throughput). fp8 weights are
    pre-scaled by 32 on the host; the 1/32 is folded into PSUM->SBUF copies.
  - Q projection and both FFN matmuls stay bf16: their error lands on the
    residual stream where fp8 blows the 2e-2 budget (measured 5e-2 all-fp8
    vs 3.2e-3 with these three in bf16).

Schedule: the softmax exp stream (~57us on the scalar engine) is the
second-longest pole after the PE, so scores for head-pair j are emitted
immediately after the K projection of tile j, with attn*V lagging one pair
(PV(j-1) fills the PE while the vector engine re-quantizes k8[j]). V(c=0)
is interleaved into the j=0 block, V(c=1) into the j=4 block (attn*V for
pairs 0-3 only reads heads 0-7 = the c=0 half of V).

Layernorm plumbing (feature-major activations, stats via ones-column
matmuls, gamma/beta folded into W1 + rank-1 W2 corrections) as in v1; the
LN column-sum matmuls are interleaved (lag-one) into the Wo/FFN2 loops and
the LN2 normalize reads the A/B broadcast PSUM directly and writes bf16.
"""

import sys
import types

import numpy as np
import ml_dtypes


def _shim_axon_hooks():
    try:
        import antenv.axon_hooks  # noqa: F401
    except Exception:
        mod = types.ModuleType("antenv.axon_hooks")
        mod.get_axon_ntff_profile_hook = lambda: None
        mod.set_axon_ntff_profile_hook = lambda h: None
        sys.modules["antenv.axon_hooks"] = mod


_shim_axon_hooks()

from concourse import bacc, mybir, tile  # noqa: E402
from concourse import bass_utils  # noqa: E402

F32 = mybir.dt.float32
F32R = mybir.dt.float32r
BF16 = mybir.dt.bfloat16
FP8 = mybir.dt.float8e4
AF = mybir.ActivationFunctionType
DR = mybir.MatmulPerfMode.DoubleRow

B, S, D, H, DH, FF = 4, 1024, 1024, 16, 64, 2048
SQ = 512
P = 128
DT = D // P
FT = FF // P
ST = S // P
NCORES = 8
EPS = 1e-6
SCALE = 1.0 / 32.0
ALPHA = 32.0

_C_BO = 0
_C_B1 = 8
_C_B2 = 24
_C_G1 = 32
_C_BE1 = 40
_C_G2 = 48
_C_BE2 = 56


def _emit(ctx, tc, aps):
    nc = tc.nc
    (xT_ap, x8_ap, wq_ap, wk_ap, wv_ap, wo_ap, w1_ap, w2_ap, consts_ap,
     ones_ap, onesrow_ap, fold_ap, yT_ap) = aps

    acts = ctx.enter_context(tc.tile_pool(name="acts", bufs=1))
    wts = ctx.enter_context(tc.tile_pool(name="wts", bufs=2))
    e2p = ctx.enter_context(tc.tile_pool(name="e2p", bufs=8))
    sc2 = ctx.enter_context(tc.tile_pool(name="sc2", bufs=2))
    sc1 = ctx.enter_context(tc.tile_pool(name="sc1", bufs=1))
    pp = ctx.enter_context(tc.tile_pool(name="pp", bufs=1, space="PSUM"))
    pss = ctx.enter_context(tc.tile_pool(name="pss", bufs=2, space="PSUM"))
    pvp = ctx.enter_context(tc.tile_pool(name="pvp", bufs=2, space="PSUM"))

    def wload(tag, src_ap, shape, dtype, nsplit=2):
        w = wts.tile(shape, dtype, tag=tag, name=tag)
        step = shape[1] // nsplit
        for q in range(nsplit):
            nc.sync.dma_start(w[:, q * step:(q + 1) * step, :],
                              src_ap[:, q * step:(q + 1) * step, :])
        return w

    # ---- input DMA: xt8 + wk0 first (K0 gates the exp stream), then Q's ----
    xt8 = acts.tile([P, DT, S], FP8, tag="xt8", name="xt8")
    for q in range(DT):
        nc.sync.dma_start(xt8[:, q:q + 1, :], x8_ap[:, q:q + 1, :])
    wk0 = wload("wka", wk_ap[:, 0], [P, DT, P], FP8)
    wqa = wload("wqa", wq_ap[:, 0], [P, DT, P], BF16)
    wqb = wload("wqb", wq_ap[:, 1], [P, DT, P], BF16)
    xtb_a = acts.tile([P, 4, SQ], BF16, tag="xtb_a", name="xtb_a")
    nc.sync.dma_start(xtb_a[:, 0:2, :], xT_ap[:, 0:2, 0:SQ])
    nc.sync.dma_start(xtb_a[:, 2:4, :], xT_ap[:, 2:4, 0:SQ])
    xtb_b = acts.tile([P, 4, SQ], BF16, tag="xtb_b", name="xtb_b")
    nc.sync.dma_start(xtb_b[:, 0:2, :], xT_ap[:, 4:6, 0:SQ])
    nc.sync.dma_start(xtb_b[:, 2:4, :], xT_ap[:, 6:8, 0:SQ])
    consts = acts.tile([P, 64], F32, tag="consts", name="consts")
    nc.sync.dma_start(consts[:], consts_ap[:])
    ones_r = acts.tile([P, 1], F32R, tag="ones", name="ones")
    nc.sync.dma_start(ones_r[:], ones_ap[:])
    ones_row = acts.tile([1, P], F32R, tag="ones_row", name="ones_row")
    nc.sync.dma_start(ones_row[:], onesrow_ap[:])
    ones_b = acts.tile([P, 1], BF16, tag="ones_b", name="ones_b")
    nc.vector.memset(ones_b[:], 1.0)
    fold = acts.tile([1, 2 * D], BF16, tag="fold", name="fold")
    nc.sync.dma_start(fold[:], fold_ap[:])

    def xlo(k):
        return xtb_a[:, k, :] if k < 4 else xtb_b[:, k - 4, :]

    # ---- attention machinery ----
    q_res = []
    q8 = acts.tile([P, DT, SQ], FP8, tag="q8", name="q8")
    vr8 = acts.tile([P, ST, H, DH + 1], FP8, tag="vr8", name="vr8")
    nc.vector.memset(vr8[:, :, :, DH:DH + 1], 1.0)
    attn8 = acts.tile([P, DT, SQ], FP8, tag="attn8", name="attn8")
    k8 = [None] * DT
    e2s = {}
    pvs = {}

    def qpair_halves(j0, wa, wb):
        """Returns two emission closures (k 0:4 and k 4:8 + copies)."""
        box = {}

        def half(lo, hi):
            if lo == 0:
                box["ps"] = pp.tile([P, 2, SQ], F32, tag="ps2", name="ps2")
            ps = box["ps"]
            for k in range(lo, hi):
                nc.tensor.matmul(ps[:, 0, :], wa[:, k, :], xlo(k),
                                 start=(k == 0), stop=(k == DT - 1))
                nc.tensor.matmul(ps[:, 1, :], wb[:, k, :], xlo(k),
                                 start=(k == 0), stop=(k == DT - 1))
            if hi == DT:
                for h in range(2):
                    j = j0 + h
                    qr = acts.tile([P, SQ], F32R, tag=f"qres{j}",
                                   name=f"qres{j}")
                    nc.scalar.copy(qr[:], ps[:, h, :])
                    q_res.append(qr)
                    nc.vector.tensor_copy(q8[:, j, :], ps[:, h, :])

        return [lambda: half(0, 4), lambda: half(4, DT)]

    def qpair(j0, wa, wb):
        for f in qpair_halves(j0, wa, wb):
            f()

    def kproj(j, w):
        ps = pp.tile([P, 2, SQ], F32, tag="ps2", name="ps2")
        for k in range(0, DT, 2):
            nc.tensor.matmul(ps[:, 0, :], w[:, k:k + 2, :],
                             xt8[:, k:k + 2, 0:SQ],
                             start=(k == 0), stop=(k == DT - 2), perf_mode=DR)
            nc.tensor.matmul(ps[:, 1, :], w[:, k:k + 2, :],
                             xt8[:, k:k + 2, SQ:S],
                             start=(k == 0), stop=(k == DT - 2), perf_mode=DR)
        kj = acts.tile([P, S], FP8, tag=f"k8{j}", name=f"k8{j}")
        nc.vector.tensor_scalar_mul(
            kj[:].rearrange("p (c q) -> p c q", c=2), ps[:], 1.0 / ALPHA)
        k8[j] = kj

    def v_pair_halves(c, st0, wv):
        """Two emission closures, one per key tile; one psum tile + copy."""
        box = {}

        def half(si):
            if si == 0:
                box["ps"] = pp.tile([P, 2, SQ], F32, tag="ps2", name="ps2")
            ps = box["ps"]
            for k in range(0, DT, 2):
                nc.tensor.matmul(
                    ps[:, si, :],
                    xt8[:, k:k + 2, (st0 + si) * P:(st0 + si + 1) * P],
                    wv[:, k:k + 2, :],
                    start=(k == 0), stop=(k == DT - 2), perf_mode=DR)
            if si == 1:
                nc.vector.tensor_scalar_mul(
                    vr8[:, st0:st0 + 2, c * 8:(c + 1) * 8, 0:DH],
                    ps[:].rearrange("p s (h d) -> p s h d", d=DH),
                    1.0 / ALPHA)

        return [lambda: half(0), lambda: half(1)]

    def scx(j, st2):
        """scores + exp for head pair (2j, 2j+1), key tiles 2*st2, 2*st2+1."""
        e2 = e2p.tile([P, 2, 2, SQ], FP8, tag="e2", name="e2")
        for sti in range(2):
            st = 2 * st2 + sti
            sl = slice(st * P, (st + 1) * P)
            ps = pss.tile([P, 2, SQ], F32, tag="pss", name="pss")
            nc.tensor.matmul(ps[:, 0, :], k8[j][0:DH, sl], q8[0:DH, j, :],
                             start=True, stop=True)
            nc.tensor.matmul(ps[:, 1, :], k8[j][DH:P, sl], q8[DH:P, j, :],
                             start=True, stop=True)
            nc.scalar.activation(e2[:, sti, :, :], ps[:], AF.Exp, scale=SCALE)
        e2s[(j, st2)] = e2

    def pv_acc(j, st2):
        if st2 == 0:
            pvs[j] = (pvp.tile([DH + 1, SQ], F32, tag="pv", name="pv"),
                      pvp.tile([DH + 1, SQ], F32, tag="pv", name="pv"))
        pv0, pv1 = pvs[j]
        e2 = e2s.pop((j, st2))
        nc.tensor.matmul(pv0[:], vr8[:, 2 * st2:2 * st2 + 2, 2 * j, :],
                         e2[:, :, 0, :], start=(st2 == 0),
                         stop=(st2 == ST // 2 - 1), perf_mode=DR)
        nc.tensor.matmul(pv1[:], vr8[:, 2 * st2:2 * st2 + 2, 2 * j + 1, :],
                         e2[:, :, 1, :], start=(st2 == 0),
                         stop=(st2 == ST // 2 - 1), perf_mode=DR)

    def pv_all(j):
        for st2 in range(4):
            pv_acc(j, st2)

    def norm(j):
        pv0, pv1 = pvs.pop(j)
        for half, pv in ((0, pv0), (1, pv1)):
            rows = slice(half * DH, half * DH + DH)
            zh = sc2.tile([1, SQ], F32, tag="zh", name="zh")
            nc.vector.tensor_copy(zh[:], pv[DH:DH + 1, :])
            iz = sc2.tile([1, SQ], F32, tag="zh", name="iz")
            nc.vector.reciprocal_approx_fast(iz[:], zh[:])
            bz = sc2.tile([DH, SQ], F32, tag="sb", name="sb")
            nc.gpsimd.partition_broadcast(bz[:], iz[:])
            nc.vector.tensor_mul(attn8[rows, j, :], pv[0:DH, :], bz[:])

    # ---- block schedule: K(j) first so exp starts ~9us in; Q pairs and V
    # groups fill the PE between the exp-gated scores/PV chains. V(c=0)
    # completes by block 2 (pv pairs 0-3 read heads 0-7), V(c=1) by block 5.
    # attn*V for pair p runs 1-2 blocks after its exps; e2 pool depth 8
    # covers the two in-flight pairs. ----
    wv0 = wload("wv", wv_ap[:, 0], [P, DT, SQ], FP8, nsplit=2)
    kproj(0, wk0)
    qpair(0, wqa, wqb)
    scx(0, 0)
    vh = v_pair_halves(0, 0, wv0)
    vh[0]()
    scx(0, 1)
    vh[1]()
    scx(0, 2)
    scx(0, 3)
    wv1 = None
    preload = {}
    for j in range(1, DT):
        w = wload("wka" if j % 2 == 0 else "wkb", wk_ap[:, j], [P, DT, P],
                  FP8, nsplit=1)
        if j == 3:
            wv1 = wload("wv", wv_ap[:, 1], [P, DT, SQ], FP8, nsplit=2)
        # filler work items (~0.9-1.8us each) to slot between the exp-gated
        # scores; c0 V pairs done by block 2, c1 by block 5
        items = []
        vg = {1: [(0, 2), (0, 4)], 2: [(0, 6)], 3: [(1, 0)], 4: [(1, 2)],
              5: [(1, 4), (1, 6)]}.get(j, [])
        for c, st in vg:
            items += v_pair_halves(c, st, wv0 if c == 0 else wv1)
        if j < 4:
            items += qpair_halves(
                2 * j,
                wload("wqa", wq_ap[:, 2 * j], [P, DT, P], BF16, nsplit=1),
                wload("wqb", wq_ap[:, 2 * j + 1], [P, DT, P], BF16,
                      nsplit=1))
        if j >= 2:
            items.append(lambda jj=j - 2: (pv_all(jj), norm(jj)))
        if j == 6:
            # prefetch the first Wo / W1 stationaries during late attention
            items.append(lambda: preload.update(
                woa=wload("woa", wo_ap[:, 0], [P, DT, P], FP8, nsplit=1),
                wob=wload("wob", wo_ap[:, 1], [P, DT, P], FP8, nsplit=1)))
        if j == 7:
            items.append(lambda: preload.update(
                w1a=wload("w1a", w1_ap[:, 0], [P, DT, P], BF16),
                w1b=wload("w1b", w1_ap[:, 1], [P, DT, P], BF16)))
        kproj(j, w)
        for s in range(4):
            scx(j, s)
            take = max(1, (len(items) + 3 - s) // (4 - s))
            for it in items[:take]:
                it()
            items = items[take:]
        for it in items:
            it()
    for j in range(DT - 2, DT):
        pv_all(j)
        norm(j)

    # ---- output projection (fp8 DoubleRow) + relu + residual + LN1 stats ----
    h1, sq1 = [], []
    h18 = acts.tile([P, DT, SQ], BF16, tag="h18", name="h18")
    ps_sum1 = pvp.tile([1, SQ], F32, tag="pv", name="ps_sum1")
    ps_sq1 = pvp.tile([1, SQ], F32, tag="pv", name="ps_sq1")

    def colsum(ps_sum, ps_sq, src, sq, j):
        nc.tensor.matmul(ps_sum[:], ones_r[:], src[:],
                         start=(j == 0), stop=(j == DT - 1))
        nc.tensor.matmul(ps_sq[:], ones_b[:], sq[:],
                         start=(j == 0), stop=(j == DT - 1))

    for j0 in range(0, DT, 2):
        wa = preload["woa"] if j0 == 0 else wload(
            "woa", wo_ap[:, j0], [P, DT, P], FP8, nsplit=1)
        wb = preload["wob"] if j0 == 0 else wload(
            "wob", wo_ap[:, j0 + 1], [P, DT, P], FP8, nsplit=1)
        ps = pss.tile([P, 2, SQ], F32, tag="pss", name="ps_wo")
        for k in range(0, DT, 2):
            nc.tensor.matmul(ps[:, 0, :], wa[:, k:k + 2, :],
                             attn8[:, k:k + 2, :],
                             start=(k == 0), stop=(k == DT - 2), perf_mode=DR)
            nc.tensor.matmul(ps[:, 1, :], wb[:, k:k + 2, :],
                             attn8[:, k:k + 2, :],
                             start=(k == 0), stop=(k == DT - 2), perf_mode=DR)
        # lag-one interleave of the LN1 column sums
        for j in (j0 - 2, j0 - 1):
            if j >= 0:
                colsum(ps_sum1, ps_sq1, h1[j], sq1[j], j)
        for h in range(2):
            j = j0 + h
            rel = sc2.tile([P, SQ], F32R, tag="u", name="rel")
            nc.scalar.activation(rel[:], ps[:, h, :], AF.Relu,
                                 bias=consts[:, _C_BO + j:_C_BO + j + 1],
                                 scale=1.0 / ALPHA)
            t = acts.tile([P, SQ], F32R, tag=f"h1{j}", name=f"h1_{j}")
            nc.vector.tensor_add(t[:], rel[:], q_res[j][:])
            h1.append(t)
            nc.scalar.copy(h18[:, j, :], t[:])
            sq = acts.tile([P, SQ], BF16, tag=f"sq1{j}", name=f"sq1_{j}")
            nc.vector.tensor_mul(sq[:], t[:], t[:])
            sq1.append(sq)
    for j in (DT - 2, DT - 1):
        colsum(ps_sum1, ps_sq1, h1[j], sq1[j], j)

    # LN1 chain (gamma/beta folded into W1 / rank-1 W2 fold)
    s_sb = sc1.tile([1, SQ], F32, tag="s0", name="s_sb")
    nc.vector.tensor_copy(s_sb[:], ps_sum1[:])
    m2 = sc1.tile([1, SQ], F32, tag="s1", name="m2")
    nc.vector.tensor_mul(m2[:], s_sb[:], s_sb[:])
    a_t = sc1.tile([1, SQ], F32, tag="s2", name="a_t")
    nc.vector.scalar_tensor_tensor(a_t[:], m2[:], 1.0 / D, ps_sq1[:],
                                   op0=mybir.AluOpType.mult,
                                   op1=mybir.AluOpType.subtract)
    eps_t = sc1.tile([1, 1], F32, tag="eps", name="eps")
    nc.vector.memset(eps_t[:], EPS)
    sd1 = sc1.tile([1, SQ], F32, tag="s1", name="sd1")
    nc.scalar.activation(sd1[:], a_t[:], AF.Sqrt, bias=eps_t[:], scale=-1.0 / D)
    rstd1 = sc1.tile([1, SQ], F32, tag="s2", name="rstd1")
    nc.vector.reciprocal_approx_fast(rstd1[:], sd1[:])
    bneg1 = sc1.tile([1, SQ], F32, tag="s3", name="bneg1")
    nc.vector.scalar_tensor_tensor(bneg1[:], s_sb[:], -1.0 / D, rstd1[:],
                                   op0=mybir.AluOpType.mult,
                                   op1=mybir.AluOpType.mult)
    negmu_r = sc1.tile([1, SQ], BF16, tag="s4", name="negmu_r")
    nc.vector.tensor_scalar_mul(negmu_r[:], s_sb[:], -1.0 / D)
    sd_r = sc1.tile([1, SQ], BF16, tag="s5", name="sd_r")
    nc.vector.tensor_copy(sd_r[:], sd1[:])
    abc_sb = sc2.tile([P, SQ], F32, tag="sb", name="abc_sb")
    nc.gpsimd.partition_broadcast(abc_sb[:], rstd1[:])
    bbc_sb = sc2.tile([P, SQ], F32, tag="zh", name="bbc_sb")
    nc.gpsimd.partition_broadcast(bbc_sb[:], bneg1[:])

    # ---- FFN1 (bf16) ----
    hid_a = acts.tile([P, DT, SQ], BF16, tag="xtb_hi", name="hid_a")
    hid_b = acts.tile([P, DT, SQ], BF16, tag="hid_b", name="hid_b")
    for f0 in range(0, FT, 2):
        wa = preload["w1a"] if f0 == 0 else wload(
            "w1a", w1_ap[:, f0], [P, DT, P], BF16)
        wb = preload["w1b"] if f0 == 0 else wload(
            "w1b", w1_ap[:, f0 + 1], [P, DT, P], BF16)
        ps = pss.tile([P, 2, SQ], F32, tag="pss", name="ps_f1")
        for k in range(DT):
            nc.tensor.matmul(ps[:, 0, :], wa[:, k, :], h18[:, k, :],
                             start=(k == 0), stop=(k == DT - 1))
            nc.tensor.matmul(ps[:, 1, :], wb[:, k, :], h18[:, k, :],
                             start=(k == 0), stop=(k == DT - 1))
        hid = hid_a if f0 < DT else hid_b
        nc.scalar.copy(hid[:, f0 % DT:f0 % DT + 2, :], ps[:])

    # real ln1 for the residual (overlaps FFN1)
    ln1 = []
    for j in range(DT):
        u = sc2.tile([P, SQ], F32, tag="u", name="u")
        nc.vector.tensor_mul(u[:], h1[j][:], abc_sb[:])
        nc.vector.tensor_add(u[:], u[:], bbc_sb[:])
        d = acts.tile([P, SQ], F32R, tag=f"ln1{j}", name=f"ln1_{j}")
        nc.scalar.activation(d[:], u[:], AF.Identity,
                             bias=consts[:, _C_BE1 + j:_C_BE1 + j + 1],
                             scale=consts[:, _C_G1 + j:_C_G1 + j + 1])
        ln1.append(d)

    # ---- FFN2 (bf16) + rank-1 LN1 fold + relu + residual + LN2 stats ----
    f2, sq2 = [], []
    ps_sum2 = pvp.tile([1, SQ], F32, tag="pv", name="ps_sum2")
    ps_sq2 = pvp.tile([1, SQ], F32, tag="pv", name="ps_sq2")
    for j in range(DT):
        w = wload("w2", w2_ap[:, j], [P, FT, P], BF16, nsplit=4)
        ps = pss.tile([P, 2, SQ], F32, tag="pss", name="ps_f2")
        for f in range(FT):
            hid = hid_a if f < DT else hid_b
            nc.tensor.matmul(ps[:, 0, :], w[:, f, :], hid[:, f % DT, :],
                             start=(f == 0), stop=False)
        nc.tensor.matmul(ps[:, 0, :], fold[0:1, j * P:(j + 1) * P],
                         negmu_r[:], start=False, stop=False)
        nc.tensor.matmul(ps[:, 0, :], fold[0:1, D + j * P:D + (j + 1) * P],
                         sd_r[:], start=False, stop=True)
        if j > 0:
            colsum(ps_sum2, ps_sq2, f2[j - 1], sq2[j - 1], j - 1)
        rel = sc2.tile([P, SQ], F32R, tag="u", name="rel2")
        nc.vector.scalar_tensor_tensor(rel[:], ps[:, 0, :], 0.0, abc_sb[:],
                                       op0=mybir.AluOpType.max,
                                       op1=mybir.AluOpType.mult)
        t = acts.tile([P, SQ], F32R, tag=f"h1{j}", name=f"f2_{j}")
        nc.vector.tensor_add(t[:], rel[:], ln1[j][:])
        f2.append(t)
        sq = acts.tile([P, SQ], BF16, tag=f"sq1{j}", name=f"sq2_{j}")
        nc.scalar.activation(sq[:], t[:], AF.Square)
        sq2.append(sq)
    colsum(ps_sum2, ps_sq2, f2[DT - 1], sq2[DT - 1], DT - 1)

    # ---- LN2 chain + normalize (writes bf16, DMA per tile) ----
    s_sb2 = sc1.tile([1, SQ], F32, tag="s0", name="s_sb2")
    nc.vector.tensor_copy(s_sb2[:], ps_sum2[:])
    m22 = sc1.tile([1, SQ], F32, tag="s1", name="m22")
    nc.vector.tensor_mul(m22[:], s_sb2[:], s_sb2[:])
    a_t2 = sc1.tile([1, SQ], F32, tag="s2", name="a_t2")
    nc.vector.scalar_tensor_tensor(a_t2[:], m22[:], 1.0 / D, ps_sq2[:],
                                   op0=mybir.AluOpType.mult,
                                   op1=mybir.AluOpType.subtract)
    sd2 = sc1.tile([1, SQ], F32, tag="s1", name="sd2")
    nc.scalar.activation(sd2[:], a_t2[:], AF.Sqrt, bias=eps_t[:],
                         scale=-1.0 / D)
    rstd2 = sc1.tile([1, SQ], F32, tag="s2", name="rstd2")
    nc.vector.reciprocal_approx_fast(rstd2[:], sd2[:])
    bneg2 = sc1.tile([1, SQ], F32, tag="s3", name="bneg2")
    nc.vector.scalar_tensor_tensor(bneg2[:], s_sb2[:], -1.0 / D, rstd2[:],
                                   op0=mybir.AluOpType.mult,
                                   op1=mybir.AluOpType.mult)
    a_r = sc1.tile([1, SQ], F32R, tag="s0", name="a_r2")
    nc.vector.tensor_copy(a_r[:], rstd2[:])
    b_r = sc1.tile([1, SQ], F32R, tag="s4", name="b_r2")
    nc.vector.tensor_copy(b_r[:], bneg2[:])
    ab = pss.tile([P, 2, SQ], F32, tag="pss", name="ab")
    nc.tensor.matmul(ab[:, 0, :], ones_row[:], a_r[:], start=True, stop=True)
    nc.tensor.matmul(ab[:, 1, :], ones_row[:], b_r[:], start=True, stop=True)
    # SBUF copies of A/B for the gpsimd-normalized tiles (gpsimd can't read
    # PSUM); the vector-normalized tiles read the PSUM broadcast directly.
    a_sb2 = sc2.tile([P, SQ], F32, tag="sb", name="a_sb2")
    nc.scalar.copy(a_sb2[:], ab[:, 0, :])
    b_sb2 = sc2.tile([P, SQ], F32, tag="zh", name="b_sb2")
    nc.scalar.copy(b_sb2[:], ab[:, 1, :])
    HQ = SQ // 2
    for j in range(DT):
        u = sc2.tile([P, SQ], F32, tag="u", name="u")
        nc.vector.tensor_mul(u[:, 0:HQ], f2[j][:, 0:HQ], ab[:, 0, 0:HQ])
        nc.gpsimd.tensor_mul(u[:, HQ:SQ], f2[j][:, HQ:SQ], a_sb2[:, HQ:SQ])
        nc.vector.tensor_add(u[:, 0:HQ], u[:, 0:HQ], ab[:, 1, 0:HQ])
        nc.gpsimd.tensor_add(u[:, HQ:SQ], u[:, HQ:SQ], b_sb2[:, HQ:SQ])
        d = acts.tile([P, SQ], BF16, tag=f"qres{j}", name=f"y_{j}")
        nc.scalar.activation(d[:], u[:], AF.Identity,
                             bias=consts[:, _C_BE2 + j:_C_BE2 + j + 1],
                             scale=consts[:, _C_G2 + j:_C_G2 + j + 1])
        nc.sync.dma_start(yT_ap[j * P:(j + 1) * P, :], d[:])


def build():
    nc = bacc.Bacc("TRN2", target_bir_lowering=False, debug=False,
                   num_devices=NCORES)
    xT_ap = nc.dram_tensor("xT", [P, DT, S], BF16, kind="ExternalInput").ap()
    x8_ap = nc.dram_tensor("xT8", [P, DT, S], FP8, kind="ExternalInput").ap()
    wq_ap = nc.dram_tensor("Wq", [P, DT, DT, P], BF16, kind="ExternalInput").ap()
    wk_ap = nc.dram_tensor("Wk", [P, DT, DT, P], FP8, kind="ExternalInput").ap()
    wv_ap = nc.dram_tensor("Wv", [P, 2, DT, SQ], FP8, kind="ExternalInput").ap()
    wo_ap = nc.dram_tensor("Wo", [P, DT, DT, P], FP8, kind="ExternalInput").ap()
    w1_ap = nc.dram_tensor("W1", [P, FT, DT, P], BF16, kind="ExternalInput").ap()
    w2_ap = nc.dram_tensor("W2", [P, DT, FT, P], BF16, kind="ExternalInput").ap()
    consts_ap = nc.dram_tensor("consts", [P, 64], F32, kind="ExternalInput").ap()
    ones_ap = nc.dram_tensor("ones", [P, 1], F32R, kind="ExternalInput").ap()
    onesrow_ap = nc.dram_tensor("ones_row", [1, P], F32R, kind="ExternalInput").ap()
    fold_ap = nc.dram_tensor("fold", [1, 2 * D], BF16, kind="ExternalInput").ap()
    yT_ap = nc.dram_tensor("yT", [D, SQ], BF16, kind="ExternalOutput").ap()
    aps = (xT_ap, x8_ap, wq_ap, wk_ap, wv_ap, wo_ap, w1_ap, w2_ap,
           consts_ap, ones_ap, onesrow_ap, fold_ap, yT_ap)
    from contextlib import ExitStack
    with tile.TileContext(nc) as tc, ExitStack() as ctx:
        _emit(ctx, tc, aps)
    nc.compile()
    return nc


_cached_nc = None


def _get_nc():
    global _cached_nc
    if _cached_nc is None:
        _cached_nc = build()
    return _cached_nc


def _to_bf16(a):
    return np.ascontiguousarray(np.asarray(a, np.float32)).astype(
        ml_dtypes.bfloat16)


def _to_fp8(a, scale):
    return np.clip(np.asarray(a, np.float32) * scale, -240.0, 240.0).astype(
        ml_dtypes.float8_e4m3)


def _prep_in_maps(x, Wq, Wk, Wv, Wo, bo, ln1_g, ln1_b, W1, b1, W2, b2,
                  ln2_g, ln2_b):
    f = np.float32
    consts = np.zeros((P, 64), f)
    consts[:, _C_BO:_C_BO + 8] = np.asarray(bo, f).reshape(8, P).T
    consts[:, _C_B1:_C_B1 + 16] = np.asarray(b1, f).reshape(16, P).T
    consts[:, _C_B2:_C_B2 + 8] = np.asarray(b2, f).reshape(8, P).T
    consts[:, _C_G1:_C_G1 + 8] = np.asarray(ln1_g, f).reshape(8, P).T
    consts[:, _C_BE1:_C_BE1 + 8] = np.asarray(ln1_b, f).reshape(8, P).T
    consts[:, _C_G2:_C_G2 + 8] = np.asarray(ln2_g, f).reshape(8, P).T
    consts[:, _C_BE2:_C_BE2 + 8] = np.asarray(ln2_b, f).reshape(8, P).T
    ones = np.ones((P, 1), f)
    ones_row = np.ones((1, P), f)
    W1f = np.asarray(W1, np.float64)
    W2f = np.asarray(W2, np.float64)
    g1v = np.asarray(ln1_g, np.float64)
    b1v = np.asarray(ln1_b, np.float64)
    c1 = np.asarray(b1, np.float64) + (b1v[:, None] * W1f).sum(axis=0)
    W1g = (g1v[:, None] * W1f).astype(f)
    w2g1 = (g1v[:, None] * W1f).sum(axis=0) @ W2f
    c2 = np.asarray(b2, np.float64) + c1 @ W2f
    fold = np.concatenate([w2g1, c2]).astype(f)[None, :]

    def pack_st(W, dtype_fn):
        # [D_in, N] -> [P, N/P, D_in/P, P] stationary tiles
        din, n = W.shape
        return np.ascontiguousarray(
            dtype_fn(np.asarray(W, f).reshape(din // P, P, n // P, P)
                     .transpose(1, 2, 0, 3)))

    shared = {
        "Wq": pack_st(np.asarray(Wq, f), _to_bf16),
        "Wk": pack_st(np.asarray(Wk, f), lambda a: _to_fp8(a, ALPHA)),
        "Wo": pack_st(np.asarray(Wo, f), lambda a: _to_fp8(a, ALPHA)),
        "W1": pack_st(W1g, _to_bf16),
        "W2": pack_st(np.asarray(W2, f), _to_bf16),
        "Wv": np.ascontiguousarray(
            _to_fp8(np.asarray(Wv, f).reshape(DT, P, 2, SQ)
                    .transpose(1, 2, 0, 3), ALPHA)),
        "consts": consts, "ones": ones, "ones_row": ones_row,
        "fold": _to_bf16(fold),
    }
    xt = np.asarray(x, f).transpose(0, 2, 1)  # [B, D, S]
    in_maps = []
    for core in range(NCORES):
        b, off = core // 2, (core % 2) * SQ
        if off == 0:
            xrot = xt[b]
        else:
            xrot = np.concatenate([xt[b][:, off:], xt[b][:, :off]], axis=1)
        xr = xrot.reshape(DT, P, S).transpose(1, 0, 2)
        in_maps.append(dict(shared, xT=np.ascontiguousarray(_to_bf16(xr)),
                            xT8=np.ascontiguousarray(_to_fp8(xr, 1.0))))
    return in_maps


def run(inputs, trace=False, tmpdir=None):
    """Run the kernel on 8 cores. Returns (y, BassKernelResults)."""
    nc = _get_nc()
    in_maps = _prep_in_maps(
        inputs["x"], inputs["Wq"], inputs["Wk"], inputs["Wv"], inputs["Wo"],
        inputs["bo"], inputs["ln1_g"], inputs["ln1_b"], inputs["W1"],
        inputs["b1"], inputs["W2"], inputs["b2"], inputs["ln2_g"],
        inputs["ln2_b"])
    try:
        res = bass_utils.run_bass_kernel_spmd(nc, in_maps, list(range(NCORES)),
                                              trace=trace, tmpdir=tmpdir)
    except Exception:
        import time as _time
        _time.sleep(2.0)
        res = bass_utils.run_bass_kernel_spmd(nc, in_maps, list(range(NCORES)),
                                              trace=trace, tmpdir=tmpdir)
    y = np.empty((B, S, D), np.float32)
    for core in range(NCORES):
        b, off = core // 2, (core % 2) * SQ
        y[b, off:off + SQ, :] = res.results[core]["yT"].astype(np.float32).T
    return y, res


def kernel(x, mask, Wq, Wk, Wv, Wo, bo, ln1_g, ln1_b, W1, b1, W2, b2,
           ln2_g, ln2_b):
    # mask is all-ones per the problem spec -> identity in the reference.
    y, _ = run(dict(x=x, Wq=Wq, Wk=Wk, Wv=Wv, Wo=Wo, bo=bo, ln1_g=ln1_g,
                    ln1_b=ln1_b, W1=W1, b1=b1, W2=W2, b2=b2, ln2_g=ln2_g,
                    ln2_b=ln2_b))
    return y
